# revision 61
# baseline (speedup 1.0000x reference)
"""BitNet MLP (nn_BitNetMLP) Trainium2 kernel — 8-core tensor-parallel over
the intermediate dimension I.

Math (reference):
  xq    = int4_absmean_quant(x)          per-token over H
  gate  = xq @ (ternary(w_gate)*wm_g).T
  up    = xq @ (ternary(w_up)*wm_u).T
  inter = int8_absmax_quant(up * relu(gate)^2)   per-token over I
  out   = inter @ (ternary(w_down)*wm_d).T

All quantized values are small integers; matmuls run with integer-valued
fp8/bf16 operands and fp32 PSUM accumulation -> exact integer arithmetic.
Scales (beta_t, w_mean, gamma_t) fold into per-token scalars applied on the
ScalarE during PSUM evacuation. Rounding = float32 magic-number trick (RNE,
matches jnp.round); clip applied post-round in magic space.

Sharding: each core holds I/8 rows of w_gate/w_up, I/8 cols of w_down, and
full x. The host passes the weights PRE-TRANSPOSED (wgT/wuT [H, IC], wdT
[IC, H]) so the kernel never needs DMA transposes for weights; xq and iq are
transposed on the TensorEngine (transpose-via-identity) instead of DMA
transposes, eliminating the xq/iq DRAM round trips and Sync-queue
serialization entirely.

Collectives: one batched AllReduce(add) of the three |w| sums (w_mean),
per-super-block AllReduce(max) of per-token gamma partials, per-super-block
ReduceScatter(add) of [T, H] output partials.

Pipeline: mm2/RS of super-block b-1 overlaps mm1 of super-block b; all
collective-dependent DMAs are dispatched from GpSimd so the Sync HWDGE
queue never head-of-line blocks on a collective.
"""

import numpy as np

# bass_utils imports antenv.axon_hooks when tracing is requested via env;
# the module is absent in this image — seed a null hook so tracing degrades
# gracefully instead of crashing.
def _seed_axon_hooks():
    import sys, types
    try:
        import antenv.axon_hooks  # noqa: F401
    except Exception:
        try:
            import antenv
        except Exception:
            return
        m = types.ModuleType("antenv.axon_hooks")
        m.get_axon_ntff_profile_hook = lambda: None
        m.set_axon_ntff_profile_hook = lambda h: None
        sys.modules["antenv.axon_hooks"] = m
        antenv.axon_hooks = m


_seed_axon_hooks()

N_CORES = 8
B, S = 2, 2048
H, I = 4096, 11008
T = B * S                    # 4096 tokens
IC = I // N_CORES            # 1376 intermediate per core
NSLAB = 11                   # ceil(1376/128) i-slabs for mm2
ICP = NSLAB * 128            # 1408 padded
TSUP = 256                   # tokens per super-block
NSUP = T // TSUP             # 8
TT = 128                     # tokens per tile
NT = T // TT                 # 32 t_tiles
NTS = TSUP // TT             # 4 t_tiles per super
HB = H // 128                # 32 h-blocks (mm1 contraction k-tiles)
HH = H // 2                  # 2048 (x and w processed in H-halves)
MAGIC = 12582912.0           # 1.5 * 2^23: float32 round-to-nearest-int trick
EPS = 1e-5
SQRT7 = float(np.sqrt(7.0))
# mm1 PSUM column chunks of IC (each fits a 2KB fp32 PSUM bank)
CH = [(0, 512), (512, 512), (1024, IC - 1024)]

_cache = {}


def _build(debug=False):
    import contextlib
    import concourse.mybir as mybir
    import concourse.tile as tile
    from concourse import bacc, bass_isa, masks

    dt = mybir.dt
    Alu = mybir.AluOpType
    Act = mybir.ActivationFunctionType

    nc = bacc.Bacc("TRN2", target_bir_lowering=False, debug=False,
                   num_devices=N_CORES)

    x_in = nc.dram_tensor("x", [T, H], dt.float32, kind="ExternalInput")
    wg_in = nc.dram_tensor("wg", [H, IC], dt.float32, kind="ExternalInput")
    wu_in = nc.dram_tensor("wu", [H, IC], dt.float32, kind="ExternalInput")
    wd_in = nc.dram_tensor("wd", [IC, H], dt.float32, kind="ExternalInput")
    sc_in = nc.dram_tensor("scales", [1, 3], dt.float32, kind="ExternalInput")
    out_ext = nc.dram_tensor("out_rs", [NSUP, TSUP // N_CORES, H], dt.float32,
                             kind="ExternalOutput")

    RG = [list(range(N_CORES))]

    with tile.TileContext(nc) as tc:
        ctx = contextlib.ExitStack()
        with ctx:
            dram = ctx.enter_context(tc.tile_pool(name="dram", bufs=1, space="DRAM"))
            wdq_d = dram.tile([ICP, H], dt.bfloat16, tag="wdq_d")
            p_d = [dram.tile([TT, IC], dt.float32, tag=f"p{t}", name=f"p_d{t}")
                   for t in range(NT)]
            part_d = [dram.tile([TSUP, H], dt.float32, tag=f"part{b}", name=f"part_d{b}")
                      for b in range(NSUP)]
            rs_d = [dram.tile([TSUP // N_CORES, H], dt.float32, tag=f"rs{b}",
                              name=f"rs_d{b}") for b in range(NSUP)]
            # last super's partials split by h-group so its ReduceScatter
            # overlaps the tail mm2 instead of serializing after it
            part7_d = [dram.tile([TSUP, 1024], dt.float32, tag=f"p7_{g}",
                                 name=f"part7_d{g}") for g in range(4)]
            rs7_d = [dram.tile([TSUP // N_CORES, 1024], dt.float32,
                               tag=f"rs7_{g}", name=f"rs7_d{g}")
                     for g in range(4)]
            ws_d1 = dram.tile([1, 2], dt.float32, tag="ws_d1")
            ws_a1 = dram.tile([1, 2], dt.float32, tag="ws_a1")
            ws_d2 = dram.tile([1, 1], dt.float32, tag="ws_d2")
            ws_a2 = dram.tile([1, 1], dt.float32, tag="ws_a2")
            dum_d = dram.tile([1, 1], dt.float32, tag="dum_d")
            dum_a = dram.tile([1, 1], dt.float32, tag="dum_a")
            gpart_d = [dram.tile([128, NTS], dt.float32, tag=f"gpart{b}",
                                 name=f"gpart_d{b}") for b in range(NSUP)]
            gall_d = [dram.tile([128, NTS], dt.float32, tag=f"gall{b}",
                                name=f"gall_d{b}") for b in range(NSUP)]

            pwbig = ctx.enter_context(tc.tile_pool(name="wbig", bufs=4))
            px = ctx.enter_context(tc.tile_pool(name="px", bufs=3))
            pxq = ctx.enter_context(tc.tile_pool(name="pxq", bufs=3))
            pbig16 = ctx.enter_context(tc.tile_pool(name="big16", bufs=2))
            piqt = ctx.enter_context(tc.tile_pool(name="piqt", bufs=2))
            pp = ctx.enter_context(tc.tile_pool(name="pp", bufs=2))
            pr = ctx.enter_context(tc.tile_pool(name="pr", bufs=2))
            piq = ctx.enter_context(tc.tile_pool(name="piq", bufs=2))
            psm = ctx.enter_context(tc.tile_pool(name="psm", bufs=1))
            pwd = ctx.enter_context(tc.tile_pool(name="pwd", bufs=4))
            pev = ctx.enter_context(tc.tile_pool(name="pev", bufs=2))
            pps = ctx.enter_context(tc.tile_pool(name="ps", bufs=8, space="PSUM"))

            # --- small persistent tiles ---
            scs = psm.tile([1, 3], dt.float32, tag="scs")
            nc.sync.dma_start(scs[:], sc_in.ap())
            sbc = psm.tile([128, 3], dt.float32, tag="sbc")
            nc.gpsimd.partition_broadcast(sbc[:], scs[:])
            wacc = psm.tile([128, 3], dt.float32, tag="wacc")
            nc.vector.memset(wacc[:], 0.0)
            # dummy AllReduce to warm the collective path during weight scan
            nc.sync.dma_start(dum_d[:], wacc[0:1, 0:1])
            nc.gpsimd.collective_compute("AllReduce", mybir.AluOpType.add,
                                         replica_groups=RG,
                                         ins=[dum_d.opt()], outs=[dum_a.opt()])
            beta_all = psm.tile([128, NT], dt.float32, tag="beta_all")
            gam_p = psm.tile([128, NT], dt.float32, tag="gam_p")
            gam = psm.tile([128, NT], dt.float32, tag="gam")
            s2 = psm.tile([128, NT], dt.float32, tag="s2")
            Dt = psm.tile([128, NT], dt.float32, tag="Dt")
            wred = psm.tile([128, 3], dt.float32, tag="wred")
            wsb = psm.tile([1, 3], dt.float32, tag="wsb")
            wsbc = psm.tile([128, 3], dt.float32, tag="wsbc")
            wmv = psm.tile([128, 3], dt.float32, tag="wmv")
            wrec = psm.tile([128, 3], dt.float32, tag="wrec")
            ident = psm.tile([128, 128], dt.bfloat16, tag="ident")
            masks.make_identity(nc, ident[:])

            # ============ x int4 quant + TensorE transpose per super ========
            def prep_x_tt(b, ti, xqT8v):
                    t = b * NTS + ti
                    t0 = t * TT
                    xh = [px.tile([128, HH], dt.float32, tag="px",
                                  name=f"xh{t}_{h}") for h in range(2)]
                    ac = [pr.tile([128, 1], dt.float32, tag="acc",
                                  name=f"ac{t}_{h}") for h in range(2)]
                    for h in range(2):
                        # x loads on the Scalar HWDGE queue: the Sync queue
                        # carries bulk weight/slab traffic that would delay them
                        nc.scalar.dma_start(xh[h][:], x_in.ap()[t0:t0 + TT,
                                                                h * HH:(h + 1) * HH])
                        nc.vector.tensor_reduce(out=ac[h][:], in_=xh[h][:],
                                                axis=mybir.AxisListType.X, op=Alu.add,
                                                apply_absolute_value=True)
                    asum = pr.tile([128, 1], dt.float32, tag="asum", name=f"as{t}")
                    nc.vector.tensor_tensor(out=asum[:], in0=ac[0][:], in1=ac[1][:],
                                            op=Alu.add)
                    nc.vector.tensor_scalar(out=beta_all[:, t:t + 1], in0=asum[:],
                                            scalar1=1.0 / H, scalar2=None,
                                            op0=Alu.mult)
                    dbe = pr.tile([128, 1], dt.float32, tag="dbe", name=f"db{t}")
                    nc.vector.tensor_scalar(out=dbe[:], in0=asum[:], scalar1=1.0 / H,
                                            scalar2=EPS, op0=Alu.mult, op1=Alu.add)
                    rbe = pr.tile([128, 1], dt.float32, tag="rbe", name=f"rb{t}")
                    nc.vector.reciprocal(rbe[:], dbe[:])
                    sbe = pr.tile([128, 1], dt.float32, tag="sbe", name=f"sb{t}")
                    nc.vector.tensor_scalar(out=sbe[:], in0=rbe[:], scalar1=SQRT7,
                                            scalar2=None, op0=Alu.mult)
                    xq = []
                    for h in range(2):
                        nc.scalar.activation(xh[h][:], xh[h][:], Act.Copy, bias=MAGIC,
                                             scale=sbe[:])
                        nc.vector.tensor_scalar(out=xh[h][:], in0=xh[h][:],
                                                scalar1=MAGIC + 7.0,
                                                scalar2=MAGIC - 8.0,
                                                op0=Alu.min, op1=Alu.max)
                        q = pxq.tile([128, HH], dt.bfloat16, tag="pxq",
                                     name=f"xq{t}_{h}")
                        nc.vector.tensor_scalar(out=q[:], in0=xh[h][:],
                                                scalar1=-MAGIC, scalar2=None,
                                                op0=Alu.add)
                        xq.append(q)
                    # TensorE transposes: 8 groups of 4 h-blocks -> xqT8
                    for g in range(8):
                        pt = pps.tile([128, 4 * TT], dt.bfloat16, tag="ps",
                                      name=f"xtp{t}_{g}")
                        for q in range(4):
                            hb = g * 4 + q
                            src = xq[hb // 16][:, (hb % 16) * 128:(hb % 16 + 1) * 128]
                            nc.tensor.transpose(pt[:, q * TT:(q + 1) * TT], src,
                                                ident[:])
                        nc.vector.tensor_copy(
                            xqT8v[:, g * 4:(g + 1) * 4, ti * TT:(ti + 1) * TT],
                            pt[:].rearrange("p (q t) -> p q t", q=4))

            def prep_x(b):
                xqT8 = pbig16.tile([128, HB * TSUP], dt.float8e4, tag="big16",
                                   name=f"xqT8_{b}")
                xqT8v = xqT8[:].rearrange("p (hb t) -> p hb t", hb=HB)
                for ti in range(NTS):
                    prep_x_tt(b, ti, xqT8v)
                return xqT8

            # x-quant of super 0 first: its DVE/ACT work fills the weight-prep
            # AllReduce latency, and nothing of it depends on weights.
            xqT8_cur = prep_x(0)

            # ================= |w| sums + batched AllReduce =================
            # weight tiles alternate between the px and pp pools: 4 ring slots
            # keep the scan/quant loops DMA-bound instead of ring-locked
            _wtile_ctr = [0]

            def w_tile(cols, name):
                _wtile_ctr[0] += 1
                if cols <= IC and _wtile_ctr[0] % 2 == 0:
                    return pp.tile([128, cols], dt.float32, tag="pp", name=name)
                return px.tile([128, cols], dt.float32, tag="px", name=name)

            def w_abs_sum(win, rows, cols, wi):
                for r0 in range(0, rows, 128):
                    rr = min(128, rows - r0)
                    for c0 in range(0, cols, HH):
                        cc = min(HH, cols - c0)
                        wt = w_tile(cc, f"wt{wi}_{r0}_{c0}")
                        nc.sync.dma_start(wt[:rr, :], win.ap()[r0:r0 + rr, c0:c0 + cc])
                        acc = pr.tile([128, 1], dt.float32, tag="acc",
                                      name=f"wacc{wi}_{r0}_{c0}")
                        nc.scalar.activation(wt[:rr, :], wt[:rr, :], Act.Abs,
                                             accum_out=acc[:rr, :])
                        nc.vector.tensor_tensor(
                            out=wacc[:rr, wi:wi + 1], in0=wacc[:rr, wi:wi + 1],
                            in1=acc[:rr, :], op=Alu.add)

            # scan gate/up first, kick their AllReduce, then scan w_down so
            # its scan overlaps the first AR's latency.
            w_abs_sum(wg_in, H, IC, 0)
            w_abs_sum(wu_in, H, IC, 1)
            nc.gpsimd.partition_all_reduce(wred[:, 0:2], wacc[:, 0:2],
                                           channels=128,
                                           reduce_op=bass_isa.ReduceOp.add)
            nc.sync.dma_start(ws_d1[:], wred[0:1, 0:2])
            nc.gpsimd.collective_compute("AllReduce", Alu.add, replica_groups=RG,
                                         ins=[ws_d1.opt()], outs=[ws_a1.opt()])
            # post-AR loads on gpsimd so the Sync queue never waits a collective
            nc.gpsimd.dma_start(wsb[:, 0:2], ws_a1[:])
            nc.gpsimd.partition_broadcast(wsbc[:, 0:2], wsb[:, 0:2])
            nc.vector.tensor_scalar(out=wmv[:, 0:2], in0=wsbc[:, 0:2],
                                    scalar1=1.0 / (I * H),
                                    scalar2=None, op0=Alu.mult)
            nc.vector.tensor_scalar(out=wred[:, 0:2], in0=wsbc[:, 0:2],
                                    scalar1=1.0 / (I * H),
                                    scalar2=EPS, op0=Alu.mult, op1=Alu.add)
            nc.vector.reciprocal(wrec[:, 0:2], wred[:, 0:2])

            # ======= ternarize gate/up -> fp8 SBUF (resident, no DRAM) ======
            # gate/up resident fp8, split into 4 h-range tiles so mm1(0) can
            # start as soon as the first quarter is quantized.
            # tile q holds h-blocks [8q, 8q+8); col ((wi*8 + hb%8)*IC + i)
            wq8s = [pwbig.tile([128, 2 * 8 * IC], dt.float8e4, tag="wbig",
                               name=f"wq8_{q}") for q in range(4)]
            # h-block-major across gate/up so mm1(0) can start after a few tiles
            for hb in range(HB):
                for wi, win in ((0, wg_in), (1, wu_in)):
                    wt = w_tile(IC, f"wq{wi}_{hb}")
                    nc.sync.dma_start(wt[:], win.ap()[hb * 128:(hb + 1) * 128, :])
                    nc.scalar.activation(wt[:], wt[:], Act.Copy,
                                         bias=MAGIC, scale=wrec[:, wi:wi + 1])
                    nc.vector.tensor_scalar(out=wt[:], in0=wt[:],
                                            scalar1=MAGIC + 1.0,
                                            scalar2=MAGIC - 1.0,
                                            op0=Alu.min, op1=Alu.max)
                    off = (wi * 8 + hb % 8) * IC
                    nc.vector.tensor_scalar(out=wq8s[hb // 8][:, off:off + IC],
                                            in0=wt[:],
                                            scalar1=-MAGIC, scalar2=None,
                                            op0=Alu.add)
            # w_down: scan + AR2 only now (its quant loads queue behind the
            # gate/up quant loads, and AR2 latency hides under them)
            w_abs_sum(wd_in, IC, H, 2)
            nc.gpsimd.partition_all_reduce(wred[:, 2:3], wacc[:, 2:3],
                                           channels=128,
                                           reduce_op=bass_isa.ReduceOp.add)
            nc.sync.dma_start(ws_d2[:], wred[0:1, 2:3])
            nc.gpsimd.collective_compute("AllReduce", Alu.add, replica_groups=RG,
                                         ins=[ws_d2.opt()], outs=[ws_a2.opt()])
            nc.gpsimd.dma_start(wsb[:, 2:3], ws_a2[:])
            nc.gpsimd.partition_broadcast(wsbc[:, 2:3], wsb[:, 2:3])
            nc.vector.tensor_scalar(out=wmv[:, 2:3], in0=wsbc[:, 2:3],
                                    scalar1=1.0 / (I * H),
                                    scalar2=None, op0=Alu.mult)
            nc.vector.tensor_scalar(out=wred[:, 2:3], in0=wsbc[:, 2:3],
                                    scalar1=1.0 / (I * H),
                                    scalar2=EPS, op0=Alu.mult, op1=Alu.add)
            nc.vector.reciprocal(wrec[:, 2:3], wred[:, 2:3])

            # ============ ternarize w_down -> bf16 DRAM [ICP, H] ============
            # software-pipelined: tile i+1's load is issued BEFORE tile i's
            # store so the sync-queue FIFO never couples the next load to the
            # previous tile's compute (which would serialize the whole loop).
            WD_CUTS = [0, 1376, 2752, 4096]
            wd_tiles = [(sl, ci) for sl in range(NSLAB) for ci in range(3)]

            def wd_load(i):
                sl, ci = wd_tiles[i]
                c0, cc = WD_CUTS[ci], WD_CUTS[ci + 1] - WD_CUTS[ci]
                rr = min(128, IC - sl * 128)
                wt = w_tile(cc, f"wqd_{sl}_{c0}")
                # scalar queue: keeps 45MB of wd reloads off the sync queue,
                # which must stay responsive for iteration-0 traffic
                nc.scalar.dma_start(wt[:rr, :cc],
                                    wd_in.ap()[sl * 128:sl * 128 + rr,
                                               c0:c0 + cc])
                return wt

            def wd_compute(i, wt):
                sl, ci = wd_tiles[i]
                c0, cc = WD_CUTS[ci], WD_CUTS[ci + 1] - WD_CUTS[ci]
                rr = min(128, IC - sl * 128)
                nc.scalar.activation(wt[:rr, :cc], wt[:rr, :cc], Act.Copy,
                                     bias=MAGIC, scale=wrec[:rr, 2:3])
                nc.vector.tensor_scalar(out=wt[:rr, :cc], in0=wt[:rr, :cc],
                                        scalar1=MAGIC + 1.0,
                                        scalar2=MAGIC - 1.0,
                                        op0=Alu.min, op1=Alu.max)
                qb = pxq.tile([128, cc], dt.bfloat16, tag="pxq",
                              name=f"wqdb_{sl}_{c0}")
                nc.vector.tensor_scalar(out=qb[:rr, :cc], in0=wt[:rr, :cc],
                                        scalar1=-MAGIC, scalar2=None,
                                        op0=Alu.add)
                if rr < 128:
                    nc.vector.memset(qb[rr:128, :cc], 0.0)
                return qb

            def wd_store(i, qb):
                sl, ci = wd_tiles[i]
                c0, cc = WD_CUTS[ci], WD_CUTS[ci + 1] - WD_CUTS[ci]
                nc.sync.dma_start(wdq_d[sl * 128:(sl + 1) * 128, c0:c0 + cc],
                                  qb[:, :cc])

            def do_wd_quant():
                wt_prev = wd_load(0)
                qb_prev = None
                for i in range(len(wd_tiles)):
                    qb = wd_compute(i, wt_prev)
                    if i + 1 < len(wd_tiles):
                        wt_prev = wd_load(i + 1)
                    if qb_prev is not None:
                        wd_store(i - 1, qb_prev)
                    qb_prev = qb
                wd_store(len(wd_tiles) - 1, qb_prev)

            # ---- scale constants (from w sums + input scales) ----
            cgg = psm.tile([128, 1], dt.float32, tag="cgg")
            nc.vector.tensor_tensor(out=cgg[:], in0=wmv[:, 0:1], in1=sbc[:, 0:1],
                                    op=Alu.mult)
            cuu = psm.tile([128, 1], dt.float32, tag="cuu")
            nc.vector.tensor_tensor(out=cuu[:], in0=wmv[:, 1:2], in1=sbc[:, 1:2],
                                    op=Alu.mult)
            cdd = psm.tile([128, 1], dt.float32, tag="cdd")
            nc.vector.tensor_tensor(out=cdd[:], in0=wmv[:, 2:3], in1=sbc[:, 2:3],
                                    op=Alu.mult)
            cb = psm.tile([128, 1], dt.float32, tag="cb")
            nc.vector.tensor_tensor(out=cb[:], in0=cgg[:], in1=cgg[:], op=Alu.mult)
            nc.vector.tensor_tensor(out=cb[:], in0=cb[:], in1=cuu[:], op=Alu.mult)

            # ===================== main pipeline ============================
            def mm1_super(b, xqT8):
                for ti in range(NTS):
                    t = b * NTS + ti
                    ghs = [pps.tile([128, cw], dt.float32, tag="ps",
                                    name=f"g{t}_{ci}") for ci, (c0, cw) in enumerate(CH)]
                    uhs = [pps.tile([128, cw], dt.float32, tag="ps",
                                    name=f"u{t}_{ci}") for ci, (c0, cw) in enumerate(CH)]
                    for k in range(HB // 2):
                        lhs = (xqT8[:, 2 * k * TSUP:(2 * k + 2) * TSUP]
                               .rearrange("p (j t) -> p j t", j=2)
                               [:, :, ti * TT:(ti + 1) * TT])
                        st, sp = (k == 0), (k == HB // 2 - 1)
                        for wi, ph in ((0, ghs), (1, uhs)):
                            hb0 = 2 * k
                            pair = (wq8s[hb0 // 8][:, (wi * 8 + hb0 % 8) * IC:
                                                   (wi * 8 + hb0 % 8 + 2) * IC]
                                    .rearrange("p (j i) -> p j i", j=2))
                            for ci, (c0, cw) in enumerate(CH):
                                nc.tensor.matmul(
                                    ph[ci][:], lhs,
                                    pair[:, :, c0:c0 + cw],
                                    start=st, stop=sp,
                                    perf_mode=mybir.MatmulPerfMode.DoubleRow)
                    pt = pp.tile([128, IC], dt.float32, tag="pp", name=f"pt{t}")
                    for ci, (c0, cw) in enumerate(CH):
                        rt = pr.tile([128, cw], dt.float32, tag="rt",
                                     name=f"rt{t}_{ci}")
                        nc.scalar.activation(rt[:], ghs[ci][:], Act.Relu)
                        nc.scalar.activation(rt[:], rt[:], Act.Square)
                        nc.vector.tensor_tensor(
                            out=pt[:, c0:c0 + cw],
                            in0=rt[:], in1=uhs[ci][:], op=Alu.mult)
                    nc.vector.tensor_reduce(out=gam_p[:, t:t + 1], in_=pt[:],
                                            axis=mybir.AxisListType.X, op=Alu.max,
                                            apply_absolute_value=True)
                    nc.sync.dma_start(p_d[t][:], pt[:])

            def gamma_ar(b):
                sl0, sl1 = b * NTS, (b + 1) * NTS
                nc.sync.dma_start(gpart_d[b][:], gam_p[:, sl0:sl1])
                nc.gpsimd.collective_compute("AllReduce", Alu.max, replica_groups=RG,
                                             ins=[gpart_d[b].opt()],
                                             outs=[gall_d[b].opt()])
                nc.gpsimd.dma_start(gam[:, sl0:sl1], gall_d[b][:])

            def gamma_scales(b):
                sl0, sl1 = b * NTS, (b + 1) * NTS
                t1 = pr.tile([128, NTS], dt.float32, tag="gsc", name=f"gs{b}a")
                nc.vector.tensor_tensor(out=t1[:], in0=beta_all[:, sl0:sl1],
                                        in1=beta_all[:, sl0:sl1], op=Alu.mult)
                nc.vector.tensor_tensor(out=t1[:], in0=t1[:],
                                        in1=beta_all[:, sl0:sl1], op=Alu.mult)
                nc.vector.tensor_scalar(out=t1[:], in0=t1[:], scalar1=cb[:],
                                        scalar2=None, op0=Alu.mult)  # Ct
                cgs = pr.tile([128, NTS], dt.float32, tag="gsc2", name=f"gs{b}b")
                nc.vector.tensor_tensor(out=cgs[:], in0=t1[:], in1=gam[:, sl0:sl1],
                                        op=Alu.mult)  # C*gam
                rn = pr.tile([128, NTS], dt.float32, tag="gsc3", name=f"gs{b}c")
                nc.vector.tensor_scalar(out=rn[:], in0=cgs[:], scalar1=EPS,
                                        scalar2=None, op0=Alu.add)
                nc.vector.reciprocal(rn[:], rn[:])
                nc.vector.tensor_scalar(out=t1[:], in0=t1[:], scalar1=127.0,
                                        scalar2=None, op0=Alu.mult)
                nc.vector.tensor_tensor(out=s2[:, sl0:sl1], in0=t1[:], in1=rn[:],
                                        op=Alu.mult)
                nc.vector.tensor_scalar(out=cgs[:], in0=cgs[:], scalar1=cdd[:],
                                        scalar2=None, op0=Alu.mult)
                nc.vector.tensor_scalar(out=Dt[:, sl0:sl1], in0=cgs[:],
                                        scalar1=1.0 / 127.0, scalar2=None,
                                        op0=Alu.mult)

            def quant_super(b, pre=()):
                iqT = piqt.tile([128, NSLAB * TSUP], dt.bfloat16, tag="piqt",
                                name=f"iqT_{b}")
                iqTv = iqT[:].rearrange("p (sb t) -> p sb t", sb=NSLAB)
                for ti in range(NTS):
                    t = b * NTS + ti
                    if ti < len(pre):
                        pt = pre[ti]
                    else:
                        pt = pp.tile([128, IC], dt.float32, tag="pp",
                                     name=f"pq{t}")
                        nc.sync.dma_start(pt[:], p_d[t][:])
                    nc.scalar.activation(pt[:], pt[:], Act.Copy, bias=MAGIC,
                                         scale=s2[:, t:t + 1])
                    nc.vector.tensor_scalar(out=pt[:], in0=pt[:],
                                            scalar1=MAGIC + 127.0,
                                            scalar2=MAGIC - 128.0,
                                            op0=Alu.min, op1=Alu.max)
                    qt = piq.tile([128, ICP], dt.bfloat16, tag="piq", name=f"qt{t}")
                    nc.vector.tensor_scalar(out=qt[:, 0:IC], in0=pt[:],
                                            scalar1=-MAGIC, scalar2=None, op0=Alu.add)
                    nc.vector.memset(qt[:, IC:ICP], 0.0)
                    # TensorE transposes: slabs in groups of <=4 -> iqT
                    for g in range(3):
                        nsb = 4 if g < 2 else NSLAB - 8
                        ptp = pps.tile([128, nsb * TT], dt.bfloat16, tag="ps",
                                       name=f"itp{t}_{g}")
                        for q in range(nsb):
                            sb = g * 4 + q
                            nc.tensor.transpose(ptp[:, q * TT:(q + 1) * TT],
                                                qt[:, sb * 128:(sb + 1) * 128],
                                                ident[:])
                        nc.vector.tensor_copy(
                            iqTv[:, g * 4:g * 4 + nsb, ti * TT:(ti + 1) * TT],
                            ptp[:].rearrange("p (q t) -> p q t", q=nsb))
                return iqT

            def mm2_super(b, iqT, px_fn=None, split_rs=False):
                for hhg in range(4):
                    ops = [[pps.tile([128, 512], dt.float32, tag="ps",
                                     name=f"o{b}_{hhg}_{ti}_{j}") for j in range(2)]
                           for ti in range(NTS)]
                    for k in range(NSLAB):
                        slab = pwd.tile([128, 1024], dt.bfloat16, tag="pwd",
                                        name=f"wds{b}_{hhg}_{k}")
                        nc.sync.dma_start(
                            slab[:], wdq_d[k * 128:(k + 1) * 128,
                                           hhg * 1024:(hhg + 1) * 1024])
                        st, sp = (k == 0), (k == NSLAB - 1)
                        for ti in range(NTS):
                            lhs = iqT[:, k * TSUP + ti * TT: k * TSUP + (ti + 1) * TT]
                            nc.tensor.matmul(ops[ti][0][:], lhs, slab[:, 0:512],
                                             start=st, stop=sp)
                            nc.tensor.matmul(ops[ti][1][:], lhs,
                                             slab[:, 512:1024], start=st, stop=sp)
                    for ti in range(NTS):
                        t = b * NTS + ti
                        ev = pev.tile([128, 1024], dt.float32, tag="pev",
                                      name=f"ev{b}_{hhg}_{ti}")
                        for j in range(2):
                            nc.scalar.activation(ev[:, j * 512:(j + 1) * 512],
                                                 ops[ti][j][:], Act.Copy,
                                                 scale=Dt[:, t:t + 1])
                        if split_rs:
                            nc.sync.dma_start(
                                part7_d[hhg][ti * TT:(ti + 1) * TT, :], ev[:])
                        else:
                            nc.sync.dma_start(
                                part_d[b][ti * TT:(ti + 1) * TT,
                                          hhg * 1024:(hhg + 1) * 1024], ev[:])
                    if split_rs:
                        nc.gpsimd.collective_compute(
                            "ReduceScatter", Alu.add, replica_groups=RG,
                            ins=[part7_d[hhg].opt()], outs=[rs7_d[hhg].opt()])
                    # interleave next super's x-prep t_tiles between hhg
                    # passes: its transposes land between matmul blocks on
                    # the Tensor queue, its loads/quant hide under the MMs
                    # (NTS t_tiles spread over the 4 hhg passes)
                    if px_fn is not None and hhg % (4 // NTS) == 0:
                        px_fn(hhg // (4 // NTS))

            def rs_super(b):
                nc.gpsimd.collective_compute("ReduceScatter", Alu.add,
                                             replica_groups=RG,
                                             ins=[part_d[b].opt()],
                                             outs=[rs_d[b].opt()])

            iqts = {}
            for b in range(NSUP):
                mm1_super(b, xqT8_cur)
                gamma_ar(b)
                # prefetch the first p-tiles for quant(b) now, ahead of the
                # slab/part bulk below: their sync-queue position makes the
                # AR(b) -> quant(b) chain start the moment gamma lands
                pq_pre = []
                for ti in range(2):
                    t = b * NTS + ti
                    pq = pp.tile([128, IC], dt.float32, tag="pp", name=f"pq{t}")
                    nc.sync.dma_start(pq[:], p_d[t][:])
                    pq_pre.append(pq)
                if b == 0:
                    # w_down quant here: earlier placement would head-of-line
                    # block gpart(0)/p-stores on sync and delay AR(0)
                    do_wd_quant()
                    xqT8_cur = prep_x(1)
                else:
                    px_fn = None
                    if b + 1 < NSUP:
                        nxt = pbig16.tile([128, HB * TSUP], dt.float8e4,
                                          tag="big16", name=f"xqT8_{b + 1}")
                        nxtv = nxt[:].rearrange("p (hb t) -> p hb t", hb=HB)

                        def px_fn(i, bb=b + 1, v=nxtv):
                            prep_x_tt(bb, i, v)

                    mm2_super(b - 1, iqts.pop(b - 1), px_fn)
                    rs_super(b - 1)
                    if b + 1 < NSUP:
                        xqT8_cur = nxt
                gamma_scales(b)
                iqts[b] = quant_super(b, pq_pre)
            mm2_super(NSUP - 1, iqts.pop(NSUP - 1), split_rs=True)
            # final output copies (RS-dependent; parked at the very end)
            for b in range(NSUP - 1):
                nc.gpsimd.dma_start(out_ext.ap()[b], rs_d[b][:])
            for g in range(4):
                nc.gpsimd.dma_start(
                    out_ext.ap()[NSUP - 1][:, g * 1024:(g + 1) * 1024],
                    rs7_d[g][:])

    nc.compile()
    return nc


def _get_compiled():
    if "nc" not in _cache:
        _cache["nc"] = _build()
    return _cache["nc"]


def _make_in_maps(x, w_gate, w_up, w_down, s_gate, s_up, s_down):
    xf = np.ascontiguousarray(np.asarray(x).reshape(T, H).astype(np.float32,
                                                                 copy=False))
    scales = np.array([[float(np.asarray(s_gate).reshape(-1)[0]),
                        float(np.asarray(s_up).reshape(-1)[0]),
                        float(np.asarray(s_down).reshape(-1)[0])]],
                      dtype=np.float32)
    in_maps = []
    for c in range(N_CORES):
        i0 = c * IC
        in_maps.append({
            "x": xf,
            "wg": np.ascontiguousarray(w_gate[i0:i0 + IC, :].T,
                                       dtype=np.float32),
            "wu": np.ascontiguousarray(w_up[i0:i0 + IC, :].T,
                                       dtype=np.float32),
            "wd": np.ascontiguousarray(w_down[:, i0:i0 + IC].T,
                                       dtype=np.float32),
            "scales": scales,
        })
    return in_maps


def _assemble_out(results):
    out = np.empty((T, H), dtype=np.float32)
    tpc = TSUP // N_CORES
    for c in range(N_CORES):
        o = results[c]["out_rs"]
        for b in range(NSUP):
            out[b * TSUP + c * tpc: b * TSUP + (c + 1) * tpc] = o[b]
    return out.reshape(B, S, H)


def kernel(x, w_gate, w_up, w_down, s_gate, s_up, s_down):
    from concourse.bass_utils import run_bass_kernel_spmd

    nc = _get_compiled()
    in_maps = _make_in_maps(x, w_gate, w_up, w_down, s_gate, s_up, s_down)
    res = run_bass_kernel_spmd(nc, in_maps, core_ids=list(range(N_CORES)))
    return _assemble_out(res.results)


# revision 64
# speedup vs baseline: 1.1235x; 1.1235x over previous
"""BitNet MLP (nn_BitNetMLP) Trainium2 kernel — 8-core tensor-parallel over
the intermediate dimension I.

Math (reference):
  xq    = int4_absmean_quant(x)          per-token over H
  gate  = xq @ (ternary(w_gate)*wm_g).T
  up    = xq @ (ternary(w_up)*wm_u).T
  inter = int8_absmax_quant(up * relu(gate)^2)   per-token over I
  out   = inter @ (ternary(w_down)*wm_d).T

All quantized values are small integers; matmuls run with integer-valued
fp8/bf16 operands and fp32 PSUM accumulation -> exact integer arithmetic.
Scales (beta_t, w_mean, gamma_t) fold into per-token scalars applied on the
ScalarE during PSUM evacuation. Rounding = float32 magic-number trick (RNE,
matches jnp.round); clip applied post-round in magic space.

Sharding: each core holds I/8 rows of w_gate/w_up, I/8 cols of w_down, and
full x. The host passes the weights PRE-TRANSPOSED (wgT/wuT [H, IC], wdT
[IC, H]) so the kernel never needs DMA transposes for weights; xq and iq are
transposed on the TensorEngine (transpose-via-identity) instead of DMA
transposes, eliminating the xq/iq DRAM round trips and Sync-queue
serialization entirely.

Collectives: one batched AllReduce(add) of the three |w| sums (w_mean),
per-super-block AllReduce(max) of per-token gamma partials, per-super-block
ReduceScatter(add) of [T, H] output partials.

Pipeline: mm2/RS of super-block b-1 overlaps mm1 of super-block b; all
collective-dependent DMAs are dispatched from GpSimd so the Sync HWDGE
queue never head-of-line blocks on a collective.
"""

import numpy as np

# bass_utils imports antenv.axon_hooks when tracing is requested via env;
# the module is absent in this image — seed a null hook so tracing degrades
# gracefully instead of crashing.
def _seed_axon_hooks():
    import sys, types
    try:
        import antenv.axon_hooks  # noqa: F401
    except Exception:
        try:
            import antenv
        except Exception:
            return
        m = types.ModuleType("antenv.axon_hooks")
        m.get_axon_ntff_profile_hook = lambda: None
        m.set_axon_ntff_profile_hook = lambda h: None
        sys.modules["antenv.axon_hooks"] = m
        antenv.axon_hooks = m


_seed_axon_hooks()

N_CORES = 8
B, S = 2, 2048
H, I = 4096, 11008
T = B * S                    # 4096 tokens
IC = I // N_CORES            # 1376 intermediate per core
NSLAB = 11                   # ceil(1376/128) i-slabs for mm2
ICP = NSLAB * 128            # 1408 padded
TSUP = 512                   # tokens per super-block
NSUP = T // TSUP             # 8
TT = 128                     # tokens per tile
NT = T // TT                 # 32 t_tiles
NTS = TSUP // TT             # 4 t_tiles per super
HB = H // 128                # 32 h-blocks (mm1 contraction k-tiles)
HH = H // 2                  # 2048 (x and w processed in H-halves)
MAGIC = 12582912.0           # 1.5 * 2^23: float32 round-to-nearest-int trick
EPS = 1e-5
SQRT7 = float(np.sqrt(7.0))
# mm1 PSUM column chunks of IC (each fits a 2KB fp32 PSUM bank)
CH = [(0, 512), (512, 512), (1024, IC - 1024)]

_cache = {}


def _build(debug=False):
    import contextlib
    import concourse.mybir as mybir
    import concourse.tile as tile
    from concourse import bacc, bass_isa, masks

    dt = mybir.dt
    Alu = mybir.AluOpType
    Act = mybir.ActivationFunctionType

    nc = bacc.Bacc("TRN2", target_bir_lowering=False, debug=False,
                   num_devices=N_CORES)

    x_in = nc.dram_tensor("x", [T, H], dt.float32, kind="ExternalInput")
    wg_in = nc.dram_tensor("wg", [H, IC], dt.float32, kind="ExternalInput")
    wu_in = nc.dram_tensor("wu", [H, IC], dt.float32, kind="ExternalInput")
    wd_in = nc.dram_tensor("wd", [IC, H], dt.float32, kind="ExternalInput")
    sc_in = nc.dram_tensor("scales", [1, 3], dt.float32, kind="ExternalInput")
    out_ext = nc.dram_tensor("out_rs", [NSUP, TSUP // N_CORES, H], dt.float32,
                             kind="ExternalOutput")

    RG = [list(range(N_CORES))]

    with tile.TileContext(nc) as tc:
        ctx = contextlib.ExitStack()
        with ctx:
            dram = ctx.enter_context(tc.tile_pool(name="dram", bufs=1, space="DRAM"))
            wdq_d = dram.tile([ICP, H], dt.bfloat16, tag="wdq_d")
            p_d = [dram.tile([TT, IC], dt.float32, tag=f"p{t}", name=f"p_d{t}")
                   for t in range(NT)]
            part_d = [dram.tile([TSUP, H], dt.float32, tag=f"part{b}", name=f"part_d{b}")
                      for b in range(NSUP)]
            rs_d = [dram.tile([TSUP // N_CORES, H], dt.float32, tag=f"rs{b}",
                              name=f"rs_d{b}") for b in range(NSUP)]
            # last super's partials split by h-group so its ReduceScatter
            # overlaps the tail mm2 instead of serializing after it
            part7_d = [dram.tile([TSUP, 1024], dt.float32, tag=f"p7_{g}",
                                 name=f"part7_d{g}") for g in range(4)]
            rs7_d = [dram.tile([TSUP // N_CORES, 1024], dt.float32,
                               tag=f"rs7_{g}", name=f"rs7_d{g}")
                     for g in range(4)]
            ws_d1 = dram.tile([1, 2], dt.float32, tag="ws_d1")
            ws_a1 = dram.tile([1, 2], dt.float32, tag="ws_a1")
            ws_d2 = dram.tile([1, 1], dt.float32, tag="ws_d2")
            ws_a2 = dram.tile([1, 1], dt.float32, tag="ws_a2")
            dum_d = dram.tile([1, 1], dt.float32, tag="dum_d")
            dum_a = dram.tile([1, 1], dt.float32, tag="dum_a")
            gpart_d = [dram.tile([128, NTS], dt.float32, tag=f"gpart{b}",
                                 name=f"gpart_d{b}") for b in range(NSUP)]
            gall_d = [dram.tile([128, NTS], dt.float32, tag=f"gall{b}",
                                name=f"gall_d{b}") for b in range(NSUP)]

            pwbig = ctx.enter_context(tc.tile_pool(name="wbig", bufs=4))
            px = ctx.enter_context(tc.tile_pool(name="px", bufs=2))
            pxq = ctx.enter_context(tc.tile_pool(name="pxq", bufs=3))
            pbig16 = ctx.enter_context(tc.tile_pool(name="big16", bufs=2))
            piqt = ctx.enter_context(tc.tile_pool(name="piqt", bufs=2))
            pp = ctx.enter_context(tc.tile_pool(name="pp", bufs=2))
            pr = ctx.enter_context(tc.tile_pool(name="pr", bufs=2))
            piq = ctx.enter_context(tc.tile_pool(name="piq", bufs=2))
            psm = ctx.enter_context(tc.tile_pool(name="psm", bufs=1))
            pwd = ctx.enter_context(tc.tile_pool(name="pwd", bufs=4))
            pev = ctx.enter_context(tc.tile_pool(name="pev", bufs=2))
            pps = ctx.enter_context(tc.tile_pool(name="ps", bufs=8, space="PSUM"))

            # --- small persistent tiles ---
            scs = psm.tile([1, 3], dt.float32, tag="scs")
            nc.sync.dma_start(scs[:], sc_in.ap())
            sbc = psm.tile([128, 3], dt.float32, tag="sbc")
            nc.gpsimd.partition_broadcast(sbc[:], scs[:])
            wacc = psm.tile([128, 3], dt.float32, tag="wacc")
            nc.vector.memset(wacc[:], 0.0)
            # dummy AllReduce to warm the collective path during weight scan
            nc.sync.dma_start(dum_d[:], wacc[0:1, 0:1])
            nc.gpsimd.collective_compute("AllReduce", mybir.AluOpType.add,
                                         replica_groups=RG,
                                         ins=[dum_d.opt()], outs=[dum_a.opt()])
            beta_all = psm.tile([128, NT], dt.float32, tag="beta_all")
            gam_p = psm.tile([128, NT], dt.float32, tag="gam_p")
            gam = psm.tile([128, NT], dt.float32, tag="gam")
            s2 = psm.tile([128, NT], dt.float32, tag="s2")
            Dt = psm.tile([128, NT], dt.float32, tag="Dt")
            wred = psm.tile([128, 3], dt.float32, tag="wred")
            wsb = psm.tile([1, 3], dt.float32, tag="wsb")
            wsbc = psm.tile([128, 3], dt.float32, tag="wsbc")
            wmv = psm.tile([128, 3], dt.float32, tag="wmv")
            wrec = psm.tile([128, 3], dt.float32, tag="wrec")
            ident = psm.tile([128, 128], dt.bfloat16, tag="ident")
            masks.make_identity(nc, ident[:])

            # ============ x int4 quant + TensorE transpose per super ========
            def prep_x_tt(b, ti, xqT8v):
                    t = b * NTS + ti
                    t0 = t * TT
                    xh = [px.tile([128, HH], dt.float32, tag="px",
                                  name=f"xh{t}_{h}") for h in range(2)]
                    ac = [pr.tile([128, 1], dt.float32, tag="acc",
                                  name=f"ac{t}_{h}") for h in range(2)]
                    for h in range(2):
                        # x loads on the Scalar HWDGE queue: the Sync queue
                        # carries bulk weight/slab traffic that would delay them
                        nc.scalar.dma_start(xh[h][:], x_in.ap()[t0:t0 + TT,
                                                                h * HH:(h + 1) * HH])
                        nc.vector.tensor_reduce(out=ac[h][:], in_=xh[h][:],
                                                axis=mybir.AxisListType.X, op=Alu.add,
                                                apply_absolute_value=True)
                    asum = pr.tile([128, 1], dt.float32, tag="asum", name=f"as{t}")
                    nc.vector.tensor_tensor(out=asum[:], in0=ac[0][:], in1=ac[1][:],
                                            op=Alu.add)
                    nc.vector.tensor_scalar(out=beta_all[:, t:t + 1], in0=asum[:],
                                            scalar1=1.0 / H, scalar2=None,
                                            op0=Alu.mult)
                    dbe = pr.tile([128, 1], dt.float32, tag="dbe", name=f"db{t}")
                    nc.vector.tensor_scalar(out=dbe[:], in0=asum[:], scalar1=1.0 / H,
                                            scalar2=EPS, op0=Alu.mult, op1=Alu.add)
                    rbe = pr.tile([128, 1], dt.float32, tag="rbe", name=f"rb{t}")
                    nc.vector.reciprocal(rbe[:], dbe[:])
                    sbe = pr.tile([128, 1], dt.float32, tag="sbe", name=f"sb{t}")
                    nc.vector.tensor_scalar(out=sbe[:], in0=rbe[:], scalar1=SQRT7,
                                            scalar2=None, op0=Alu.mult)
                    xq = []
                    for h in range(2):
                        nc.scalar.activation(xh[h][:], xh[h][:], Act.Copy, bias=MAGIC,
                                             scale=sbe[:])
                        nc.vector.tensor_scalar(out=xh[h][:], in0=xh[h][:],
                                                scalar1=MAGIC + 7.0,
                                                scalar2=MAGIC - 8.0,
                                                op0=Alu.min, op1=Alu.max)
                        q = pxq.tile([128, HH], dt.bfloat16, tag="pxq",
                                     name=f"xq{t}_{h}")
                        nc.vector.tensor_scalar(out=q[:], in0=xh[h][:],
                                                scalar1=-MAGIC, scalar2=None,
                                                op0=Alu.add)
                        xq.append(q)
                    # TensorE transposes: 8 groups of 4 h-blocks -> xqT8
                    for g in range(8):
                        pt = pps.tile([128, 4 * TT], dt.bfloat16, tag="ps",
                                      name=f"xtp{t}_{g}")
                        for q in range(4):
                            hb = g * 4 + q
                            src = xq[hb // 16][:, (hb % 16) * 128:(hb % 16 + 1) * 128]
                            nc.tensor.transpose(pt[:, q * TT:(q + 1) * TT], src,
                                                ident[:])
                        nc.vector.tensor_copy(
                            xqT8v[:, g * 4:(g + 1) * 4, ti * TT:(ti + 1) * TT],
                            pt[:].rearrange("p (q t) -> p q t", q=4))

            def prep_x(b):
                xqT8 = pbig16.tile([128, HB * TSUP], dt.float8e4, tag="big16",
                                   name=f"xqT8_{b}")
                xqT8v = xqT8[:].rearrange("p (hb t) -> p hb t", hb=HB)
                for ti in range(NTS):
                    prep_x_tt(b, ti, xqT8v)
                return xqT8

            # x-quant of super 0 first: its DVE/ACT work fills the weight-prep
            # AllReduce latency, and nothing of it depends on weights.
            xqT8_cur = prep_x(0)

            # ================= |w| sums + batched AllReduce =================
            # weight tiles alternate between the px and pp pools: 4 ring slots
            # keep the scan/quant loops DMA-bound instead of ring-locked
            _wtile_ctr = [0]

            def w_tile(cols, name):
                _wtile_ctr[0] += 1
                if cols <= IC and _wtile_ctr[0] % 2 == 0:
                    return pp.tile([128, cols], dt.float32, tag="pp", name=name)
                return px.tile([128, cols], dt.float32, tag="px", name=name)

            # weight loads alternate between the two HWDGE queues (sync and
            # scalar) so descriptor issue is not single-queue serialized
            def w_load_eng():
                return nc.sync if _wtile_ctr[0] % 2 == 0 else nc.scalar

            def w_abs_sum(win, rows, cols, wi):
                for r0 in range(0, rows, 128):
                    rr = min(128, rows - r0)
                    for c0 in range(0, cols, HH):
                        cc = min(HH, cols - c0)
                        eng = w_load_eng()
                        wt = w_tile(cc, f"wt{wi}_{r0}_{c0}")
                        eng.dma_start(wt[:rr, :], win.ap()[r0:r0 + rr, c0:c0 + cc])
                        acc = pr.tile([128, 1], dt.float32, tag="acc",
                                      name=f"wacc{wi}_{r0}_{c0}")
                        nc.scalar.activation(wt[:rr, :], wt[:rr, :], Act.Abs,
                                             accum_out=acc[:rr, :])
                        nc.vector.tensor_tensor(
                            out=wacc[:rr, wi:wi + 1], in0=wacc[:rr, wi:wi + 1],
                            in1=acc[:rr, :], op=Alu.add)

            # scan gate/up first, kick their AllReduce, then scan w_down so
            # its scan overlaps the first AR's latency.
            w_abs_sum(wg_in, H, IC, 0)
            w_abs_sum(wu_in, H, IC, 1)
            nc.gpsimd.partition_all_reduce(wred[:, 0:2], wacc[:, 0:2],
                                           channels=128,
                                           reduce_op=bass_isa.ReduceOp.add)
            nc.sync.dma_start(ws_d1[:], wred[0:1, 0:2])
            nc.gpsimd.collective_compute("AllReduce", Alu.add, replica_groups=RG,
                                         ins=[ws_d1.opt()], outs=[ws_a1.opt()])
            # post-AR loads on gpsimd so the Sync queue never waits a collective
            nc.gpsimd.dma_start(wsb[:, 0:2], ws_a1[:])
            nc.gpsimd.partition_broadcast(wsbc[:, 0:2], wsb[:, 0:2])
            nc.vector.tensor_scalar(out=wmv[:, 0:2], in0=wsbc[:, 0:2],
                                    scalar1=1.0 / (I * H),
                                    scalar2=None, op0=Alu.mult)
            nc.vector.tensor_scalar(out=wred[:, 0:2], in0=wsbc[:, 0:2],
                                    scalar1=1.0 / (I * H),
                                    scalar2=EPS, op0=Alu.mult, op1=Alu.add)
            nc.vector.reciprocal(wrec[:, 0:2], wred[:, 0:2])

            # ======= ternarize gate/up -> fp8 SBUF (resident, no DRAM) ======
            # gate/up resident fp8, split into 4 h-range tiles so mm1(0) can
            # start as soon as the first quarter is quantized.
            # tile q holds h-blocks [8q, 8q+8); col ((wi*8 + hb%8)*IC + i)
            wq8s = [pwbig.tile([128, 2 * 8 * IC], dt.float8e4, tag="wbig",
                               name=f"wq8_{q}") for q in range(4)]
            # h-block-major across gate/up so mm1(0) can start after a few tiles
            for hb in range(HB):
                for wi, win in ((0, wg_in), (1, wu_in)):
                    eng = w_load_eng()
                    wt = w_tile(IC, f"wq{wi}_{hb}")
                    eng.dma_start(wt[:], win.ap()[hb * 128:(hb + 1) * 128, :])
                    nc.scalar.activation(wt[:], wt[:], Act.Copy,
                                         bias=MAGIC, scale=wrec[:, wi:wi + 1])
                    nc.vector.tensor_scalar(out=wt[:], in0=wt[:],
                                            scalar1=MAGIC + 1.0,
                                            scalar2=MAGIC - 1.0,
                                            op0=Alu.min, op1=Alu.max)
                    off = (wi * 8 + hb % 8) * IC
                    nc.vector.tensor_scalar(out=wq8s[hb // 8][:, off:off + IC],
                                            in0=wt[:],
                                            scalar1=-MAGIC, scalar2=None,
                                            op0=Alu.add)
            # w_down: scan + AR2 only now (its quant loads queue behind the
            # gate/up quant loads, and AR2 latency hides under them)
            w_abs_sum(wd_in, IC, H, 2)
            nc.gpsimd.partition_all_reduce(wred[:, 2:3], wacc[:, 2:3],
                                           channels=128,
                                           reduce_op=bass_isa.ReduceOp.add)
            nc.sync.dma_start(ws_d2[:], wred[0:1, 2:3])
            nc.gpsimd.collective_compute("AllReduce", Alu.add, replica_groups=RG,
                                         ins=[ws_d2.opt()], outs=[ws_a2.opt()])
            nc.gpsimd.dma_start(wsb[:, 2:3], ws_a2[:])
            nc.gpsimd.partition_broadcast(wsbc[:, 2:3], wsb[:, 2:3])
            nc.vector.tensor_scalar(out=wmv[:, 2:3], in0=wsbc[:, 2:3],
                                    scalar1=1.0 / (I * H),
                                    scalar2=None, op0=Alu.mult)
            nc.vector.tensor_scalar(out=wred[:, 2:3], in0=wsbc[:, 2:3],
                                    scalar1=1.0 / (I * H),
                                    scalar2=EPS, op0=Alu.mult, op1=Alu.add)
            nc.vector.reciprocal(wrec[:, 2:3], wred[:, 2:3])

            # ============ ternarize w_down -> bf16 DRAM [ICP, H] ============
            # software-pipelined: tile i+1's load is issued BEFORE tile i's
            # store so the sync-queue FIFO never couples the next load to the
            # previous tile's compute (which would serialize the whole loop).
            WD_CUTS = [0, 1376, 2752, 4096]
            wd_tiles = [(sl, ci) for sl in range(NSLAB) for ci in range(3)]

            def wd_load(i):
                sl, ci = wd_tiles[i]
                c0, cc = WD_CUTS[ci], WD_CUTS[ci + 1] - WD_CUTS[ci]
                rr = min(128, IC - sl * 128)
                wt = w_tile(cc, f"wqd_{sl}_{c0}")
                # scalar queue: keeps 45MB of wd reloads off the sync queue,
                # which must stay responsive for iteration-0 traffic
                nc.scalar.dma_start(wt[:rr, :cc],
                                    wd_in.ap()[sl * 128:sl * 128 + rr,
                                               c0:c0 + cc])
                return wt

            def wd_compute(i, wt):
                sl, ci = wd_tiles[i]
                c0, cc = WD_CUTS[ci], WD_CUTS[ci + 1] - WD_CUTS[ci]
                rr = min(128, IC - sl * 128)
                nc.scalar.activation(wt[:rr, :cc], wt[:rr, :cc], Act.Copy,
                                     bias=MAGIC, scale=wrec[:rr, 2:3])
                nc.vector.tensor_scalar(out=wt[:rr, :cc], in0=wt[:rr, :cc],
                                        scalar1=MAGIC + 1.0,
                                        scalar2=MAGIC - 1.0,
                                        op0=Alu.min, op1=Alu.max)
                qb = pxq.tile([128, cc], dt.bfloat16, tag="pxq",
                              name=f"wqdb_{sl}_{c0}")
                nc.vector.tensor_scalar(out=qb[:rr, :cc], in0=wt[:rr, :cc],
                                        scalar1=-MAGIC, scalar2=None,
                                        op0=Alu.add)
                if rr < 128:
                    nc.vector.memset(qb[rr:128, :cc], 0.0)
                return qb

            def wd_store(i, qb):
                sl, ci = wd_tiles[i]
                c0, cc = WD_CUTS[ci], WD_CUTS[ci + 1] - WD_CUTS[ci]
                nc.sync.dma_start(wdq_d[sl * 128:(sl + 1) * 128, c0:c0 + cc],
                                  qb[:, :cc])

            def do_wd_quant():
                wt_prev = wd_load(0)
                qb_prev = None
                for i in range(len(wd_tiles)):
                    qb = wd_compute(i, wt_prev)
                    if i + 1 < len(wd_tiles):
                        wt_prev = wd_load(i + 1)
                    if qb_prev is not None:
                        wd_store(i - 1, qb_prev)
                    qb_prev = qb
                wd_store(len(wd_tiles) - 1, qb_prev)

            # ---- scale constants (from w sums + input scales) ----
            cgg = psm.tile([128, 1], dt.float32, tag="cgg")
            nc.vector.tensor_tensor(out=cgg[:], in0=wmv[:, 0:1], in1=sbc[:, 0:1],
                                    op=Alu.mult)
            cuu = psm.tile([128, 1], dt.float32, tag="cuu")
            nc.vector.tensor_tensor(out=cuu[:], in0=wmv[:, 1:2], in1=sbc[:, 1:2],
                                    op=Alu.mult)
            cdd = psm.tile([128, 1], dt.float32, tag="cdd")
            nc.vector.tensor_tensor(out=cdd[:], in0=wmv[:, 2:3], in1=sbc[:, 2:3],
                                    op=Alu.mult)
            cb = psm.tile([128, 1], dt.float32, tag="cb")
            nc.vector.tensor_tensor(out=cb[:], in0=cgg[:], in1=cgg[:], op=Alu.mult)
            nc.vector.tensor_tensor(out=cb[:], in0=cb[:], in1=cuu[:], op=Alu.mult)

            # ===================== main pipeline ============================
            def mm1_super(b, xqT8):
                for ti in range(NTS):
                    t = b * NTS + ti
                    ghs = [pps.tile([128, cw], dt.float32, tag="ps",
                                    name=f"g{t}_{ci}") for ci, (c0, cw) in enumerate(CH)]
                    uhs = [pps.tile([128, cw], dt.float32, tag="ps",
                                    name=f"u{t}_{ci}") for ci, (c0, cw) in enumerate(CH)]
                    for k in range(HB // 2):
                        lhs = (xqT8[:, 2 * k * TSUP:(2 * k + 2) * TSUP]
                               .rearrange("p (j t) -> p j t", j=2)
                               [:, :, ti * TT:(ti + 1) * TT])
                        st, sp = (k == 0), (k == HB // 2 - 1)
                        for wi, ph in ((0, ghs), (1, uhs)):
                            hb0 = 2 * k
                            pair = (wq8s[hb0 // 8][:, (wi * 8 + hb0 % 8) * IC:
                                                   (wi * 8 + hb0 % 8 + 2) * IC]
                                    .rearrange("p (j i) -> p j i", j=2))
                            for ci, (c0, cw) in enumerate(CH):
                                nc.tensor.matmul(
                                    ph[ci][:], lhs,
                                    pair[:, :, c0:c0 + cw],
                                    start=st, stop=sp,
                                    perf_mode=mybir.MatmulPerfMode.DoubleRow)
                    pt = pp.tile([128, IC], dt.float32, tag="pp", name=f"pt{t}")
                    for ci, (c0, cw) in enumerate(CH):
                        rt = pr.tile([128, cw], dt.float32, tag="rt",
                                     name=f"rt{t}_{ci}")
                        nc.scalar.activation(rt[:], ghs[ci][:], Act.Relu)
                        nc.scalar.activation(rt[:], rt[:], Act.Square)
                        nc.vector.tensor_tensor(
                            out=pt[:, c0:c0 + cw],
                            in0=rt[:], in1=uhs[ci][:], op=Alu.mult)
                    nc.vector.tensor_reduce(out=gam_p[:, t:t + 1], in_=pt[:],
                                            axis=mybir.AxisListType.X, op=Alu.max,
                                            apply_absolute_value=True)
                    nc.sync.dma_start(p_d[t][:], pt[:])

            def gamma_ar(b):
                sl0, sl1 = b * NTS, (b + 1) * NTS
                nc.sync.dma_start(gpart_d[b][:], gam_p[:, sl0:sl1])
                nc.gpsimd.collective_compute("AllReduce", Alu.max, replica_groups=RG,
                                             ins=[gpart_d[b].opt()],
                                             outs=[gall_d[b].opt()])
                nc.gpsimd.dma_start(gam[:, sl0:sl1], gall_d[b][:])

            def gamma_scales(b):
                sl0, sl1 = b * NTS, (b + 1) * NTS
                t1 = pr.tile([128, NTS], dt.float32, tag="gsc", name=f"gs{b}a")
                nc.vector.tensor_tensor(out=t1[:], in0=beta_all[:, sl0:sl1],
                                        in1=beta_all[:, sl0:sl1], op=Alu.mult)
                nc.vector.tensor_tensor(out=t1[:], in0=t1[:],
                                        in1=beta_all[:, sl0:sl1], op=Alu.mult)
                nc.vector.tensor_scalar(out=t1[:], in0=t1[:], scalar1=cb[:],
                                        scalar2=None, op0=Alu.mult)  # Ct
                cgs = pr.tile([128, NTS], dt.float32, tag="gsc2", name=f"gs{b}b")
                nc.vector.tensor_tensor(out=cgs[:], in0=t1[:], in1=gam[:, sl0:sl1],
                                        op=Alu.mult)  # C*gam
                rn = pr.tile([128, NTS], dt.float32, tag="gsc3", name=f"gs{b}c")
                nc.vector.tensor_scalar(out=rn[:], in0=cgs[:], scalar1=EPS,
                                        scalar2=None, op0=Alu.add)
                nc.vector.reciprocal(rn[:], rn[:])
                nc.vector.tensor_scalar(out=t1[:], in0=t1[:], scalar1=127.0,
                                        scalar2=None, op0=Alu.mult)
                nc.vector.tensor_tensor(out=s2[:, sl0:sl1], in0=t1[:], in1=rn[:],
                                        op=Alu.mult)
                nc.vector.tensor_scalar(out=cgs[:], in0=cgs[:], scalar1=cdd[:],
                                        scalar2=None, op0=Alu.mult)
                nc.vector.tensor_scalar(out=Dt[:, sl0:sl1], in0=cgs[:],
                                        scalar1=1.0 / 127.0, scalar2=None,
                                        op0=Alu.mult)

            def quant_super(b, pre=()):
                iqT = piqt.tile([128, NSLAB * TSUP], dt.bfloat16, tag="piqt",
                                name=f"iqT_{b}")
                iqTv = iqT[:].rearrange("p (sb t) -> p sb t", sb=NSLAB)
                for ti in range(NTS):
                    t = b * NTS + ti
                    if ti < len(pre):
                        pt = pre[ti]
                    else:
                        pt = pp.tile([128, IC], dt.float32, tag="pp",
                                     name=f"pq{t}")
                        nc.sync.dma_start(pt[:], p_d[t][:])
                    nc.scalar.activation(pt[:], pt[:], Act.Copy, bias=MAGIC,
                                         scale=s2[:, t:t + 1])
                    nc.vector.tensor_scalar(out=pt[:], in0=pt[:],
                                            scalar1=MAGIC + 127.0,
                                            scalar2=MAGIC - 128.0,
                                            op0=Alu.min, op1=Alu.max)
                    qt = piq.tile([128, ICP], dt.bfloat16, tag="piq", name=f"qt{t}")
                    nc.vector.tensor_scalar(out=qt[:, 0:IC], in0=pt[:],
                                            scalar1=-MAGIC, scalar2=None, op0=Alu.add)
                    nc.vector.memset(qt[:, IC:ICP], 0.0)
                    # TensorE transposes: slabs in groups of <=4 -> iqT
                    for g in range(3):
                        nsb = 4 if g < 2 else NSLAB - 8
                        ptp = pps.tile([128, nsb * TT], dt.bfloat16, tag="ps",
                                       name=f"itp{t}_{g}")
                        for q in range(nsb):
                            sb = g * 4 + q
                            nc.tensor.transpose(ptp[:, q * TT:(q + 1) * TT],
                                                qt[:, sb * 128:(sb + 1) * 128],
                                                ident[:])
                        nc.vector.tensor_copy(
                            iqTv[:, g * 4:g * 4 + nsb, ti * TT:(ti + 1) * TT],
                            ptp[:].rearrange("p (q t) -> p q t", q=nsb))
                return iqT

            def mm2_super(b, iqT, px_fn=None, split_rs=False):
                for hhg in range(4):
                    ops = [[pps.tile([128, 512], dt.float32, tag="ps",
                                     name=f"o{b}_{hhg}_{ti}_{j}") for j in range(2)]
                           for ti in range(NTS)]
                    for k in range(NSLAB):
                        slab = pwd.tile([128, 1024], dt.bfloat16, tag="pwd",
                                        name=f"wds{b}_{hhg}_{k}")
                        nc.sync.dma_start(
                            slab[:], wdq_d[k * 128:(k + 1) * 128,
                                           hhg * 1024:(hhg + 1) * 1024])
                        st, sp = (k == 0), (k == NSLAB - 1)
                        for ti in range(NTS):
                            lhs = iqT[:, k * TSUP + ti * TT: k * TSUP + (ti + 1) * TT]
                            nc.tensor.matmul(ops[ti][0][:], lhs, slab[:, 0:512],
                                             start=st, stop=sp)
                            nc.tensor.matmul(ops[ti][1][:], lhs,
                                             slab[:, 512:1024], start=st, stop=sp)
                    for ti in range(NTS):
                        t = b * NTS + ti
                        ev = pev.tile([128, 1024], dt.float32, tag="pev",
                                      name=f"ev{b}_{hhg}_{ti}")
                        for j in range(2):
                            nc.scalar.activation(ev[:, j * 512:(j + 1) * 512],
                                                 ops[ti][j][:], Act.Copy,
                                                 scale=Dt[:, t:t + 1])
                        if split_rs:
                            nc.sync.dma_start(
                                part7_d[hhg][ti * TT:(ti + 1) * TT, :], ev[:])
                        else:
                            nc.sync.dma_start(
                                part_d[b][ti * TT:(ti + 1) * TT,
                                          hhg * 1024:(hhg + 1) * 1024], ev[:])
                    if split_rs:
                        nc.gpsimd.collective_compute(
                            "ReduceScatter", Alu.add, replica_groups=RG,
                            ins=[part7_d[hhg].opt()], outs=[rs7_d[hhg].opt()])
                    # interleave next super's x-prep t_tile between hhg
                    # passes: its transposes land between matmul blocks on
                    # the Tensor queue, its loads/quant hide under the MMs
                    if px_fn is not None:
                        px_fn(hhg)

            def rs_super(b):
                nc.gpsimd.collective_compute("ReduceScatter", Alu.add,
                                             replica_groups=RG,
                                             ins=[part_d[b].opt()],
                                             outs=[rs_d[b].opt()])

            iqts = {}
            for b in range(NSUP):
                mm1_super(b, xqT8_cur)
                gamma_ar(b)
                # prefetch the first p-tiles for quant(b) now, ahead of the
                # slab/part bulk below: their sync-queue position makes the
                # AR(b) -> quant(b) chain start the moment gamma lands
                pq_pre = []
                for ti in range(2):
                    t = b * NTS + ti
                    pq = pp.tile([128, IC], dt.float32, tag="pp", name=f"pq{t}")
                    nc.sync.dma_start(pq[:], p_d[t][:])
                    pq_pre.append(pq)
                if b == 0:
                    # w_down quant here: earlier placement would head-of-line
                    # block gpart(0)/p-stores on sync and delay AR(0)
                    do_wd_quant()
                    xqT8_cur = prep_x(1)
                else:
                    px_fn = None
                    if b + 1 < NSUP:
                        nxt = pbig16.tile([128, HB * TSUP], dt.float8e4,
                                          tag="big16", name=f"xqT8_{b + 1}")
                        nxtv = nxt[:].rearrange("p (hb t) -> p hb t", hb=HB)

                        def px_fn(i, bb=b + 1, v=nxtv):
                            prep_x_tt(bb, i, v)

                    mm2_super(b - 1, iqts.pop(b - 1), px_fn)
                    rs_super(b - 1)
                    if b + 1 < NSUP:
                        xqT8_cur = nxt
                gamma_scales(b)
                iqts[b] = quant_super(b, pq_pre)
            mm2_super(NSUP - 1, iqts.pop(NSUP - 1), split_rs=True)
            # final output copies (RS-dependent; parked at the very end)
            for b in range(NSUP - 1):
                nc.gpsimd.dma_start(out_ext.ap()[b], rs_d[b][:])
            for g in range(4):
                nc.gpsimd.dma_start(
                    out_ext.ap()[NSUP - 1][:, g * 1024:(g + 1) * 1024],
                    rs7_d[g][:])

    nc.compile()
    return nc


def _get_compiled():
    if "nc" not in _cache:
        _cache["nc"] = _build()
    return _cache["nc"]


def _make_in_maps(x, w_gate, w_up, w_down, s_gate, s_up, s_down):
    xf = np.ascontiguousarray(np.asarray(x).reshape(T, H).astype(np.float32,
                                                                 copy=False))
    scales = np.array([[float(np.asarray(s_gate).reshape(-1)[0]),
                        float(np.asarray(s_up).reshape(-1)[0]),
                        float(np.asarray(s_down).reshape(-1)[0])]],
                      dtype=np.float32)
    in_maps = []
    for c in range(N_CORES):
        i0 = c * IC
        in_maps.append({
            "x": xf,
            "wg": np.ascontiguousarray(w_gate[i0:i0 + IC, :].T,
                                       dtype=np.float32),
            "wu": np.ascontiguousarray(w_up[i0:i0 + IC, :].T,
                                       dtype=np.float32),
            "wd": np.ascontiguousarray(w_down[:, i0:i0 + IC].T,
                                       dtype=np.float32),
            "scales": scales,
        })
    return in_maps


def _assemble_out(results):
    out = np.empty((T, H), dtype=np.float32)
    tpc = TSUP // N_CORES
    for c in range(N_CORES):
        o = results[c]["out_rs"]
        for b in range(NSUP):
            out[b * TSUP + c * tpc: b * TSUP + (c + 1) * tpc] = o[b]
    return out.reshape(B, S, H)


def kernel(x, w_gate, w_up, w_down, s_gate, s_up, s_down):
    from concourse.bass_utils import run_bass_kernel_spmd

    nc = _get_compiled()
    in_maps = _make_in_maps(x, w_gate, w_up, w_down, s_gate, s_up, s_down)
    res = run_bass_kernel_spmd(nc, in_maps, core_ids=list(range(N_CORES)))
    return _assemble_out(res.results)


# revision 71
# speedup vs baseline: 1.1486x; 1.0223x over previous
"""BitNet MLP (nn_BitNetMLP) Trainium2 kernel — 8-core tensor-parallel over
the intermediate dimension I.

Math (reference):
  xq    = int4_absmean_quant(x)          per-token over H
  gate  = xq @ (ternary(w_gate)*wm_g).T
  up    = xq @ (ternary(w_up)*wm_u).T
  inter = int8_absmax_quant(up * relu(gate)^2)   per-token over I
  out   = inter @ (ternary(w_down)*wm_d).T

All quantized values are small integers; matmuls run with integer-valued
fp8/bf16 operands and fp32 PSUM accumulation -> exact integer arithmetic.
Scales (beta_t, w_mean, gamma_t) fold into per-token scalars applied on the
ScalarE during PSUM evacuation. Rounding = float32 magic-number trick (RNE,
matches jnp.round); clip applied post-round in magic space.

Sharding: each core holds I/8 rows of w_gate/w_up, I/8 cols of w_down, and
full x. The host passes the weights PRE-TRANSPOSED (wgT/wuT [H, IC], wdT
[IC, H]) so the kernel never needs DMA transposes for weights; xq and iq are
transposed on the TensorEngine (transpose-via-identity) instead of DMA
transposes, eliminating the xq/iq DRAM round trips and Sync-queue
serialization entirely.

Collectives: one batched AllReduce(add) of the three |w| sums (w_mean),
per-super-block AllReduce(max) of per-token gamma partials, per-super-block
ReduceScatter(add) of [T, H] output partials.

Pipeline: mm2/RS of super-block b-1 overlaps mm1 of super-block b; all
collective-dependent DMAs are dispatched from GpSimd so the Sync HWDGE
queue never head-of-line blocks on a collective.
"""

import numpy as np

# bass_utils imports antenv.axon_hooks when tracing is requested via env;
# the module is absent in this image — seed a null hook so tracing degrades
# gracefully instead of crashing.
def _seed_axon_hooks():
    import sys, types
    try:
        import antenv.axon_hooks  # noqa: F401
    except Exception:
        try:
            import antenv
        except Exception:
            return
        m = types.ModuleType("antenv.axon_hooks")
        m.get_axon_ntff_profile_hook = lambda: None
        m.set_axon_ntff_profile_hook = lambda h: None
        sys.modules["antenv.axon_hooks"] = m
        antenv.axon_hooks = m


_seed_axon_hooks()

N_CORES = 8
B, S = 2, 2048
H, I = 4096, 11008
T = B * S                    # 4096 tokens
IC = I // N_CORES            # 1376 intermediate per core
NSLAB = 11                   # ceil(1376/128) i-slabs for mm2
ICP = NSLAB * 128            # 1408 padded
TSUP = 512                   # tokens per super-block
NSUP = T // TSUP             # 8
TT = 128                     # tokens per tile
NT = T // TT                 # 32 t_tiles
NTS = TSUP // TT             # 4 t_tiles per super
HB = H // 128                # 32 h-blocks (mm1 contraction k-tiles)
HH = H // 2                  # 2048 (x and w processed in H-halves)
MAGIC = 12582912.0           # 1.5 * 2^23: float32 round-to-nearest-int trick
EPS = 1e-5
SQRT7 = float(np.sqrt(7.0))
# mm1 PSUM column chunks of IC (each fits a 2KB fp32 PSUM bank)
CH = [(0, 512), (512, 512), (1024, IC - 1024)]

_cache = {}


def _build(debug=False):
    import contextlib
    import concourse.mybir as mybir
    import concourse.tile as tile
    from concourse import bacc, bass_isa, masks

    dt = mybir.dt
    Alu = mybir.AluOpType
    Act = mybir.ActivationFunctionType

    nc = bacc.Bacc("TRN2", target_bir_lowering=False, debug=False,
                   num_devices=N_CORES)

    x_in = nc.dram_tensor("x", [T, H], dt.float32, kind="ExternalInput")
    wg_in = nc.dram_tensor("wg", [H, IC], dt.float32, kind="ExternalInput")
    wu_in = nc.dram_tensor("wu", [H, IC], dt.float32, kind="ExternalInput")
    wd_in = nc.dram_tensor("wd", [IC, H], dt.float32, kind="ExternalInput")
    sc_in = nc.dram_tensor("scales", [1, 3], dt.float32, kind="ExternalInput")
    out_ext = nc.dram_tensor("out_rs", [NSUP, TSUP // N_CORES, H], dt.float32,
                             kind="ExternalOutput")

    RG = [list(range(N_CORES))]

    with tile.TileContext(nc) as tc:
        ctx = contextlib.ExitStack()
        with ctx:
            dram = ctx.enter_context(tc.tile_pool(name="dram", bufs=1, space="DRAM"))
            wdq_d = dram.tile([ICP, H], dt.bfloat16, tag="wdq_d")
            p_d = [dram.tile([TT, IC], dt.float32, tag=f"p{t}", name=f"p_d{t}")
                   for t in range(NT)]
            part_d = [dram.tile([TSUP, H], dt.float32, tag=f"part{b}", name=f"part_d{b}")
                      for b in range(NSUP)]
            rs_d = [dram.tile([TSUP // N_CORES, H], dt.float32, tag=f"rs{b}",
                              name=f"rs_d{b}") for b in range(NSUP)]
            # last super's partials split by h-group so its ReduceScatter
            # overlaps the tail mm2 instead of serializing after it
            part7_d = [dram.tile([TSUP, 1024], dt.float32, tag=f"p7_{g}",
                                 name=f"part7_d{g}") for g in range(4)]
            rs7_d = [dram.tile([TSUP // N_CORES, 1024], dt.float32,
                               tag=f"rs7_{g}", name=f"rs7_d{g}")
                     for g in range(4)]
            ws_d1 = dram.tile([1, 2], dt.float32, tag="ws_d1")
            ws_a1 = dram.tile([1, 2], dt.float32, tag="ws_a1")
            ws_d2 = dram.tile([1, 1], dt.float32, tag="ws_d2")
            ws_a2 = dram.tile([1, 1], dt.float32, tag="ws_a2")
            dum_d = dram.tile([1, 1], dt.float32, tag="dum_d")
            dum_a = dram.tile([1, 1], dt.float32, tag="dum_a")
            gpart_d = [dram.tile([128, NTS], dt.float32, tag=f"gpart{b}",
                                 name=f"gpart_d{b}") for b in range(NSUP)]
            gall_d = [dram.tile([128, NTS], dt.float32, tag=f"gall{b}",
                                name=f"gall_d{b}") for b in range(NSUP)]

            pwbig = ctx.enter_context(tc.tile_pool(name="wbig", bufs=4))
            px = ctx.enter_context(tc.tile_pool(name="px", bufs=2))
            pxq = ctx.enter_context(tc.tile_pool(name="pxq", bufs=3))
            pbig16 = ctx.enter_context(tc.tile_pool(name="big16", bufs=2))
            piqt = ctx.enter_context(tc.tile_pool(name="piqt", bufs=2))
            pp = ctx.enter_context(tc.tile_pool(name="pp", bufs=2))
            pr = ctx.enter_context(tc.tile_pool(name="pr", bufs=2))
            piq = ctx.enter_context(tc.tile_pool(name="piq", bufs=2))
            psm = ctx.enter_context(tc.tile_pool(name="psm", bufs=1))
            pwd = ctx.enter_context(tc.tile_pool(name="pwd", bufs=4))
            pev = ctx.enter_context(tc.tile_pool(name="pev", bufs=2))
            pps = ctx.enter_context(tc.tile_pool(name="ps", bufs=8, space="PSUM"))

            # --- small persistent tiles ---
            scs = psm.tile([1, 3], dt.float32, tag="scs")
            nc.sync.dma_start(scs[:], sc_in.ap())
            sbc = psm.tile([128, 3], dt.float32, tag="sbc")
            nc.gpsimd.partition_broadcast(sbc[:], scs[:])
            wacc = psm.tile([128, 3], dt.float32, tag="wacc")
            nc.vector.memset(wacc[:], 0.0)
            # dummy AllReduce to warm the collective path during weight scan
            nc.sync.dma_start(dum_d[:], wacc[0:1, 0:1])
            nc.gpsimd.collective_compute("AllReduce", mybir.AluOpType.add,
                                         replica_groups=RG,
                                         ins=[dum_d.opt()], outs=[dum_a.opt()])
            beta_all = psm.tile([128, NT], dt.float32, tag="beta_all")
            gam_p = psm.tile([128, NT], dt.float32, tag="gam_p")
            gam = psm.tile([128, NT], dt.float32, tag="gam")
            s2 = psm.tile([128, NT], dt.float32, tag="s2")
            Dt = psm.tile([128, NT], dt.float32, tag="Dt")
            wred = psm.tile([128, 3], dt.float32, tag="wred")
            wsb = psm.tile([1, 3], dt.float32, tag="wsb")
            wsbc = psm.tile([128, 3], dt.float32, tag="wsbc")
            wmv = psm.tile([128, 3], dt.float32, tag="wmv")
            wrec = psm.tile([128, 3], dt.float32, tag="wrec")
            ident = psm.tile([128, 128], dt.bfloat16, tag="ident")
            masks.make_identity(nc, ident[:])

            # ============ x int4 quant + TensorE transpose per super ========
            def prep_x_tt(b, ti, xqT8v):
                    t = b * NTS + ti
                    t0 = t * TT
                    xh = [px.tile([128, HH], dt.float32, tag="px",
                                  name=f"xh{t}_{h}") for h in range(2)]
                    ac = [pr.tile([128, 1], dt.float32, tag="acc",
                                  name=f"ac{t}_{h}") for h in range(2)]
                    for h in range(2):
                        # x loads on the Scalar HWDGE queue: the Sync queue
                        # carries bulk weight/slab traffic that would delay them
                        nc.scalar.dma_start(xh[h][:], x_in.ap()[t0:t0 + TT,
                                                                h * HH:(h + 1) * HH])
                        nc.vector.tensor_reduce(out=ac[h][:], in_=xh[h][:],
                                                axis=mybir.AxisListType.X, op=Alu.add,
                                                apply_absolute_value=True)
                    asum = pr.tile([128, 1], dt.float32, tag="asum", name=f"as{t}")
                    nc.vector.tensor_tensor(out=asum[:], in0=ac[0][:], in1=ac[1][:],
                                            op=Alu.add)
                    nc.vector.tensor_scalar(out=beta_all[:, t:t + 1], in0=asum[:],
                                            scalar1=1.0 / H, scalar2=None,
                                            op0=Alu.mult)
                    dbe = pr.tile([128, 1], dt.float32, tag="dbe", name=f"db{t}")
                    nc.vector.tensor_scalar(out=dbe[:], in0=asum[:], scalar1=1.0 / H,
                                            scalar2=EPS, op0=Alu.mult, op1=Alu.add)
                    rbe = pr.tile([128, 1], dt.float32, tag="rbe", name=f"rb{t}")
                    nc.vector.reciprocal(rbe[:], dbe[:])
                    sbe = pr.tile([128, 1], dt.float32, tag="sbe", name=f"sb{t}")
                    nc.vector.tensor_scalar(out=sbe[:], in0=rbe[:], scalar1=SQRT7,
                                            scalar2=None, op0=Alu.mult)
                    xq = []
                    for h in range(2):
                        nc.scalar.activation(xh[h][:], xh[h][:], Act.Copy, bias=MAGIC,
                                             scale=sbe[:])
                        nc.vector.tensor_scalar(out=xh[h][:], in0=xh[h][:],
                                                scalar1=MAGIC + 7.0,
                                                scalar2=MAGIC - 8.0,
                                                op0=Alu.min, op1=Alu.max)
                        q = pxq.tile([128, HH], dt.bfloat16, tag="pxq",
                                     name=f"xq{t}_{h}")
                        nc.vector.tensor_scalar(out=q[:], in0=xh[h][:],
                                                scalar1=-MAGIC, scalar2=None,
                                                op0=Alu.add)
                        xq.append(q)
                    # TensorE transposes: 8 groups of 4 h-blocks -> xqT8
                    for g in range(8):
                        pt = pps.tile([128, 4 * TT], dt.bfloat16, tag="ps",
                                      name=f"xtp{t}_{g}")
                        for q in range(4):
                            hb = g * 4 + q
                            src = xq[hb // 16][:, (hb % 16) * 128:(hb % 16 + 1) * 128]
                            nc.tensor.transpose(pt[:, q * TT:(q + 1) * TT], src,
                                                ident[:])
                        nc.vector.tensor_copy(
                            xqT8v[:, g * 4:(g + 1) * 4, ti * TT:(ti + 1) * TT],
                            pt[:].rearrange("p (q t) -> p q t", q=4))

            def prep_x(b):
                xqT8 = pbig16.tile([128, HB * TSUP], dt.float8e4, tag="big16",
                                   name=f"xqT8_{b}")
                xqT8v = xqT8[:].rearrange("p (hb t) -> p hb t", hb=HB)
                for ti in range(NTS):
                    prep_x_tt(b, ti, xqT8v)
                return xqT8

            # x-quant of super 0 first: its DVE/ACT work fills the weight-prep
            # AllReduce latency, and nothing of it depends on weights.
            xqT8_cur = prep_x(0)

            # ================= |w| sums + batched AllReduce =================
            # weight tiles alternate between the px and pp pools: 4 ring slots
            # keep the scan/quant loops DMA-bound instead of ring-locked
            _wtile_ctr = [0]

            def w_tile(cols, name):
                _wtile_ctr[0] += 1
                if cols <= IC and _wtile_ctr[0] % 2 == 0:
                    return pp.tile([128, cols], dt.float32, tag="pp", name=name)
                return px.tile([128, cols], dt.float32, tag="px", name=name)

            def w_abs_sum(win, rows, cols, wi):
                for r0 in range(0, rows, 128):
                    rr = min(128, rows - r0)
                    for c0 in range(0, cols, HH):
                        cc = min(HH, cols - c0)
                        wt = w_tile(cc, f"wt{wi}_{r0}_{c0}")
                        nc.sync.dma_start(wt[:rr, :], win.ap()[r0:r0 + rr, c0:c0 + cc])
                        acc = pr.tile([128, 1], dt.float32, tag="acc",
                                      name=f"wacc{wi}_{r0}_{c0}")
                        nc.scalar.activation(wt[:rr, :], wt[:rr, :], Act.Abs,
                                             accum_out=acc[:rr, :])
                        nc.vector.tensor_tensor(
                            out=wacc[:rr, wi:wi + 1], in0=wacc[:rr, wi:wi + 1],
                            in1=acc[:rr, :], op=Alu.add)

            # scan gate/up first, kick their AllReduce, then scan w_down so
            # its scan overlaps the first AR's latency.
            w_abs_sum(wg_in, H, IC, 0)
            w_abs_sum(wu_in, H, IC, 1)
            nc.gpsimd.partition_all_reduce(wred[:, 0:2], wacc[:, 0:2],
                                           channels=128,
                                           reduce_op=bass_isa.ReduceOp.add)
            nc.sync.dma_start(ws_d1[:], wred[0:1, 0:2])
            nc.gpsimd.collective_compute("AllReduce", Alu.add, replica_groups=RG,
                                         ins=[ws_d1.opt()], outs=[ws_a1.opt()])
            # post-AR loads on gpsimd so the Sync queue never waits a collective
            nc.gpsimd.dma_start(wsb[:, 0:2], ws_a1[:])
            nc.gpsimd.partition_broadcast(wsbc[:, 0:2], wsb[:, 0:2])
            nc.vector.tensor_scalar(out=wmv[:, 0:2], in0=wsbc[:, 0:2],
                                    scalar1=1.0 / (I * H),
                                    scalar2=None, op0=Alu.mult)
            nc.vector.tensor_scalar(out=wred[:, 0:2], in0=wsbc[:, 0:2],
                                    scalar1=1.0 / (I * H),
                                    scalar2=EPS, op0=Alu.mult, op1=Alu.add)
            nc.vector.reciprocal(wrec[:, 0:2], wred[:, 0:2])

            # ======= ternarize gate/up -> fp8 SBUF (resident, no DRAM) ======
            # gate/up resident fp8, split into 4 h-range tiles so mm1(0) can
            # start as soon as the first quarter is quantized.
            # tile q holds h-blocks [8q, 8q+8); col ((wi*8 + hb%8)*IC + i)
            wq8s = [pwbig.tile([128, 2 * 8 * IC], dt.float8e4, tag="wbig",
                               name=f"wq8_{q}") for q in range(4)]
            # h-block-major across gate/up so mm1(0) can start after a few tiles
            for hb in range(HB):
                for wi, win in ((0, wg_in), (1, wu_in)):
                    wt = w_tile(IC, f"wq{wi}_{hb}")
                    nc.sync.dma_start(wt[:], win.ap()[hb * 128:(hb + 1) * 128, :])
                    nc.scalar.activation(wt[:], wt[:], Act.Copy,
                                         bias=MAGIC, scale=wrec[:, wi:wi + 1])
                    nc.vector.tensor_scalar(out=wt[:], in0=wt[:],
                                            scalar1=MAGIC + 1.0,
                                            scalar2=MAGIC - 1.0,
                                            op0=Alu.min, op1=Alu.max)
                    off = (wi * 8 + hb % 8) * IC
                    nc.vector.tensor_scalar(out=wq8s[hb // 8][:, off:off + IC],
                                            in0=wt[:],
                                            scalar1=-MAGIC, scalar2=None,
                                            op0=Alu.add)
            # w_down: scan + AR2 only now (its quant loads queue behind the
            # gate/up quant loads, and AR2 latency hides under them)
            w_abs_sum(wd_in, IC, H, 2)
            nc.gpsimd.partition_all_reduce(wred[:, 2:3], wacc[:, 2:3],
                                           channels=128,
                                           reduce_op=bass_isa.ReduceOp.add)
            nc.sync.dma_start(ws_d2[:], wred[0:1, 2:3])
            nc.gpsimd.collective_compute("AllReduce", Alu.add, replica_groups=RG,
                                         ins=[ws_d2.opt()], outs=[ws_a2.opt()])
            nc.gpsimd.dma_start(wsb[:, 2:3], ws_a2[:])
            nc.gpsimd.partition_broadcast(wsbc[:, 2:3], wsb[:, 2:3])
            nc.vector.tensor_scalar(out=wmv[:, 2:3], in0=wsbc[:, 2:3],
                                    scalar1=1.0 / (I * H),
                                    scalar2=None, op0=Alu.mult)
            nc.vector.tensor_scalar(out=wred[:, 2:3], in0=wsbc[:, 2:3],
                                    scalar1=1.0 / (I * H),
                                    scalar2=EPS, op0=Alu.mult, op1=Alu.add)
            nc.vector.reciprocal(wrec[:, 2:3], wred[:, 2:3])

            # ============ ternarize w_down -> bf16 DRAM [ICP, H] ============
            # software-pipelined: tile i+1's load is issued BEFORE tile i's
            # store so the sync-queue FIFO never couples the next load to the
            # previous tile's compute (which would serialize the whole loop).
            WD_CUTS = [0, 1376, 2752, 4096]
            wd_tiles = [(sl, ci) for sl in range(NSLAB) for ci in range(3)]

            def wd_load(i):
                sl, ci = wd_tiles[i]
                c0, cc = WD_CUTS[ci], WD_CUTS[ci + 1] - WD_CUTS[ci]
                rr = min(128, IC - sl * 128)
                wt = w_tile(cc, f"wqd_{sl}_{c0}")
                # scalar queue: keeps 45MB of wd reloads off the sync queue,
                # which must stay responsive for iteration-0 traffic
                nc.scalar.dma_start(wt[:rr, :cc],
                                    wd_in.ap()[sl * 128:sl * 128 + rr,
                                               c0:c0 + cc])
                return wt

            def wd_compute(i, wt):
                sl, ci = wd_tiles[i]
                c0, cc = WD_CUTS[ci], WD_CUTS[ci + 1] - WD_CUTS[ci]
                rr = min(128, IC - sl * 128)
                nc.scalar.activation(wt[:rr, :cc], wt[:rr, :cc], Act.Copy,
                                     bias=MAGIC, scale=wrec[:rr, 2:3])
                nc.vector.tensor_scalar(out=wt[:rr, :cc], in0=wt[:rr, :cc],
                                        scalar1=MAGIC + 1.0,
                                        scalar2=MAGIC - 1.0,
                                        op0=Alu.min, op1=Alu.max)
                qb = pxq.tile([128, cc], dt.bfloat16, tag="pxq",
                              name=f"wqdb_{sl}_{c0}")
                nc.vector.tensor_scalar(out=qb[:rr, :cc], in0=wt[:rr, :cc],
                                        scalar1=-MAGIC, scalar2=None,
                                        op0=Alu.add)
                if rr < 128:
                    nc.vector.memset(qb[rr:128, :cc], 0.0)
                return qb

            def wd_store(i, qb):
                sl, ci = wd_tiles[i]
                c0, cc = WD_CUTS[ci], WD_CUTS[ci + 1] - WD_CUTS[ci]
                nc.sync.dma_start(wdq_d[sl * 128:(sl + 1) * 128, c0:c0 + cc],
                                  qb[:, :cc])

            def do_wd_quant():
                wt_prev = wd_load(0)
                qb_prev = None
                for i in range(len(wd_tiles)):
                    qb = wd_compute(i, wt_prev)
                    if i + 1 < len(wd_tiles):
                        wt_prev = wd_load(i + 1)
                    if qb_prev is not None:
                        wd_store(i - 1, qb_prev)
                    qb_prev = qb
                wd_store(len(wd_tiles) - 1, qb_prev)

            # ---- scale constants (from w sums + input scales) ----
            cgg = psm.tile([128, 1], dt.float32, tag="cgg")
            nc.vector.tensor_tensor(out=cgg[:], in0=wmv[:, 0:1], in1=sbc[:, 0:1],
                                    op=Alu.mult)
            cuu = psm.tile([128, 1], dt.float32, tag="cuu")
            nc.vector.tensor_tensor(out=cuu[:], in0=wmv[:, 1:2], in1=sbc[:, 1:2],
                                    op=Alu.mult)
            cdd = psm.tile([128, 1], dt.float32, tag="cdd")
            nc.vector.tensor_tensor(out=cdd[:], in0=wmv[:, 2:3], in1=sbc[:, 2:3],
                                    op=Alu.mult)
            cb = psm.tile([128, 1], dt.float32, tag="cb")
            nc.vector.tensor_tensor(out=cb[:], in0=cgg[:], in1=cgg[:], op=Alu.mult)
            nc.vector.tensor_tensor(out=cb[:], in0=cb[:], in1=cuu[:], op=Alu.mult)

            # ===================== main pipeline ============================
            def mm1_super(b, xqT8):
                for ti in range(NTS):
                    t = b * NTS + ti
                    ghs = [pps.tile([128, cw], dt.float32, tag="ps",
                                    name=f"g{t}_{ci}") for ci, (c0, cw) in enumerate(CH)]
                    uhs = [pps.tile([128, cw], dt.float32, tag="ps",
                                    name=f"u{t}_{ci}") for ci, (c0, cw) in enumerate(CH)]
                    for k in range(HB // 2):
                        lhs = (xqT8[:, 2 * k * TSUP:(2 * k + 2) * TSUP]
                               .rearrange("p (j t) -> p j t", j=2)
                               [:, :, ti * TT:(ti + 1) * TT])
                        st, sp = (k == 0), (k == HB // 2 - 1)
                        for wi, ph in ((0, ghs), (1, uhs)):
                            hb0 = 2 * k
                            pair = (wq8s[hb0 // 8][:, (wi * 8 + hb0 % 8) * IC:
                                                   (wi * 8 + hb0 % 8 + 2) * IC]
                                    .rearrange("p (j i) -> p j i", j=2))
                            for ci, (c0, cw) in enumerate(CH):
                                nc.tensor.matmul(
                                    ph[ci][:], lhs,
                                    pair[:, :, c0:c0 + cw],
                                    start=st, stop=sp,
                                    perf_mode=mybir.MatmulPerfMode.DoubleRow)
                    pt = pp.tile([128, IC], dt.float32, tag="pp", name=f"pt{t}")
                    for ci, (c0, cw) in enumerate(CH):
                        rt = pr.tile([128, cw], dt.float32, tag="rt",
                                     name=f"rt{t}_{ci}")
                        nc.scalar.activation(rt[:], ghs[ci][:], Act.Relu)
                        nc.scalar.activation(rt[:], rt[:], Act.Square)
                        nc.vector.tensor_tensor(
                            out=pt[:, c0:c0 + cw],
                            in0=rt[:], in1=uhs[ci][:], op=Alu.mult)
                    nc.vector.tensor_reduce(out=gam_p[:, t:t + 1], in_=pt[:],
                                            axis=mybir.AxisListType.X, op=Alu.max,
                                            apply_absolute_value=True)
                    nc.sync.dma_start(p_d[t][:], pt[:])

            def gamma_ar(b):
                sl0, sl1 = b * NTS, (b + 1) * NTS
                nc.sync.dma_start(gpart_d[b][:], gam_p[:, sl0:sl1])
                nc.gpsimd.collective_compute("AllReduce", Alu.max, replica_groups=RG,
                                             ins=[gpart_d[b].opt()],
                                             outs=[gall_d[b].opt()])
                nc.gpsimd.dma_start(gam[:, sl0:sl1], gall_d[b][:])

            def gamma_scales(b):
                sl0, sl1 = b * NTS, (b + 1) * NTS
                t1 = pr.tile([128, NTS], dt.float32, tag="gsc", name=f"gs{b}a")
                nc.vector.tensor_tensor(out=t1[:], in0=beta_all[:, sl0:sl1],
                                        in1=beta_all[:, sl0:sl1], op=Alu.mult)
                nc.vector.tensor_tensor(out=t1[:], in0=t1[:],
                                        in1=beta_all[:, sl0:sl1], op=Alu.mult)
                nc.vector.tensor_scalar(out=t1[:], in0=t1[:], scalar1=cb[:],
                                        scalar2=None, op0=Alu.mult)  # Ct
                cgs = pr.tile([128, NTS], dt.float32, tag="gsc2", name=f"gs{b}b")
                nc.vector.tensor_tensor(out=cgs[:], in0=t1[:], in1=gam[:, sl0:sl1],
                                        op=Alu.mult)  # C*gam
                rn = pr.tile([128, NTS], dt.float32, tag="gsc3", name=f"gs{b}c")
                nc.vector.tensor_scalar(out=rn[:], in0=cgs[:], scalar1=EPS,
                                        scalar2=None, op0=Alu.add)
                nc.vector.reciprocal(rn[:], rn[:])
                nc.vector.tensor_scalar(out=t1[:], in0=t1[:], scalar1=127.0,
                                        scalar2=None, op0=Alu.mult)
                nc.vector.tensor_tensor(out=s2[:, sl0:sl1], in0=t1[:], in1=rn[:],
                                        op=Alu.mult)
                nc.vector.tensor_scalar(out=cgs[:], in0=cgs[:], scalar1=cdd[:],
                                        scalar2=None, op0=Alu.mult)
                nc.vector.tensor_scalar(out=Dt[:, sl0:sl1], in0=cgs[:],
                                        scalar1=1.0 / 127.0, scalar2=None,
                                        op0=Alu.mult)

            def quant_super(b, pre=()):
                iqT = piqt.tile([128, NSLAB * TSUP], dt.bfloat16, tag="piqt",
                                name=f"iqT_{b}")
                iqTv = iqT[:].rearrange("p (sb t) -> p sb t", sb=NSLAB)
                for ti in range(NTS):
                    t = b * NTS + ti
                    if ti < len(pre):
                        pt = pre[ti]
                    else:
                        pt = pp.tile([128, IC], dt.float32, tag="pp",
                                     name=f"pq{t}")
                        nc.sync.dma_start(pt[:], p_d[t][:])
                    nc.scalar.activation(pt[:], pt[:], Act.Copy, bias=MAGIC,
                                         scale=s2[:, t:t + 1])
                    nc.vector.tensor_scalar(out=pt[:], in0=pt[:],
                                            scalar1=MAGIC + 127.0,
                                            scalar2=MAGIC - 128.0,
                                            op0=Alu.min, op1=Alu.max)
                    qt = piq.tile([128, ICP], dt.bfloat16, tag="piq", name=f"qt{t}")
                    nc.vector.tensor_scalar(out=qt[:, 0:IC], in0=pt[:],
                                            scalar1=-MAGIC, scalar2=None, op0=Alu.add)
                    nc.vector.memset(qt[:, IC:ICP], 0.0)
                    # TensorE transposes: slabs in groups of <=4 -> iqT
                    for g in range(3):
                        nsb = 4 if g < 2 else NSLAB - 8
                        ptp = pps.tile([128, nsb * TT], dt.bfloat16, tag="ps",
                                       name=f"itp{t}_{g}")
                        for q in range(nsb):
                            sb = g * 4 + q
                            nc.tensor.transpose(ptp[:, q * TT:(q + 1) * TT],
                                                qt[:, sb * 128:(sb + 1) * 128],
                                                ident[:])
                        nc.vector.tensor_copy(
                            iqTv[:, g * 4:g * 4 + nsb, ti * TT:(ti + 1) * TT],
                            ptp[:].rearrange("p (q t) -> p q t", q=nsb))
                return iqT

            def mm2_super(b, iqT, px_fn=None, split_rs=False):
                for hhg in range(4):
                    ops = [[pps.tile([128, 512], dt.float32, tag="ps",
                                     name=f"o{b}_{hhg}_{ti}_{j}") for j in range(2)]
                           for ti in range(NTS)]
                    for k in range(NSLAB):
                        slab = pwd.tile([128, 1024], dt.bfloat16, tag="pwd",
                                        name=f"wds{b}_{hhg}_{k}")
                        nc.sync.dma_start(
                            slab[:], wdq_d[k * 128:(k + 1) * 128,
                                           hhg * 1024:(hhg + 1) * 1024])
                        st, sp = (k == 0), (k == NSLAB - 1)
                        for ti in range(NTS):
                            lhs = iqT[:, k * TSUP + ti * TT: k * TSUP + (ti + 1) * TT]
                            nc.tensor.matmul(ops[ti][0][:], lhs, slab[:, 0:512],
                                             start=st, stop=sp)
                            nc.tensor.matmul(ops[ti][1][:], lhs,
                                             slab[:, 512:1024], start=st, stop=sp)
                    for ti in range(NTS):
                        t = b * NTS + ti
                        ev = pev.tile([128, 1024], dt.float32, tag="pev",
                                      name=f"ev{b}_{hhg}_{ti}")
                        for j in range(2):
                            nc.scalar.activation(ev[:, j * 512:(j + 1) * 512],
                                                 ops[ti][j][:], Act.Copy,
                                                 scale=Dt[:, t:t + 1])
                        if split_rs:
                            nc.sync.dma_start(
                                part7_d[hhg][ti * TT:(ti + 1) * TT, :], ev[:])
                        else:
                            nc.sync.dma_start(
                                part_d[b][ti * TT:(ti + 1) * TT,
                                          hhg * 1024:(hhg + 1) * 1024], ev[:])
                    if split_rs:
                        nc.gpsimd.collective_compute(
                            "ReduceScatter", Alu.add, replica_groups=RG,
                            ins=[part7_d[hhg].opt()], outs=[rs7_d[hhg].opt()])
                    # interleave next super's x-prep t_tile between hhg
                    # passes: its transposes land between matmul blocks on
                    # the Tensor queue, its loads/quant hide under the MMs
                    if px_fn is not None:
                        px_fn(hhg)

            def rs_super(b):
                nc.gpsimd.collective_compute("ReduceScatter", Alu.add,
                                             replica_groups=RG,
                                             ins=[part_d[b].opt()],
                                             outs=[rs_d[b].opt()])

            iqts = {}
            for b in range(NSUP):
                mm1_super(b, xqT8_cur)
                gamma_ar(b)
                # prefetch the first p-tiles for quant(b) now, ahead of the
                # slab/part bulk below: their sync-queue position makes the
                # AR(b) -> quant(b) chain start the moment gamma lands
                pq_pre = []
                for ti in range(2):
                    t = b * NTS + ti
                    pq = pp.tile([128, IC], dt.float32, tag="pp", name=f"pq{t}")
                    nc.sync.dma_start(pq[:], p_d[t][:])
                    pq_pre.append(pq)
                if b == 0:
                    # w_down quant here: earlier placement would head-of-line
                    # block gpart(0)/p-stores on sync and delay AR(0)
                    do_wd_quant()
                    xqT8_cur = prep_x(1)
                else:
                    px_fn = None
                    if b + 1 < NSUP:
                        nxt = pbig16.tile([128, HB * TSUP], dt.float8e4,
                                          tag="big16", name=f"xqT8_{b + 1}")
                        nxtv = nxt[:].rearrange("p (hb t) -> p hb t", hb=HB)

                        def px_fn(i, bb=b + 1, v=nxtv):
                            prep_x_tt(bb, i, v)

                    mm2_super(b - 1, iqts.pop(b - 1), px_fn)
                    rs_super(b - 1)
                    # drain finished output copies early: RS(b-3) completed
                    # two supers ago, so this never blocks the gpsimd queue
                    if b >= 3:
                        nc.gpsimd.dma_start(out_ext.ap()[b - 3], rs_d[b - 3][:])
                    if b + 1 < NSUP:
                        xqT8_cur = nxt
                gamma_scales(b)
                iqts[b] = quant_super(b, pq_pre)
            mm2_super(NSUP - 1, iqts.pop(NSUP - 1), split_rs=True)
            # remaining output copies (supers 0..NSUP-4 drained in-loop)
            for b in range(NSUP - 3, NSUP - 1):
                nc.gpsimd.dma_start(out_ext.ap()[b], rs_d[b][:])
            for g in range(4):
                nc.gpsimd.dma_start(
                    out_ext.ap()[NSUP - 1][:, g * 1024:(g + 1) * 1024],
                    rs7_d[g][:])

    nc.compile()
    return nc


def _get_compiled():
    if "nc" not in _cache:
        _cache["nc"] = _build()
    return _cache["nc"]


def _make_in_maps(x, w_gate, w_up, w_down, s_gate, s_up, s_down):
    xf = np.ascontiguousarray(np.asarray(x).reshape(T, H).astype(np.float32,
                                                                 copy=False))
    scales = np.array([[float(np.asarray(s_gate).reshape(-1)[0]),
                        float(np.asarray(s_up).reshape(-1)[0]),
                        float(np.asarray(s_down).reshape(-1)[0])]],
                      dtype=np.float32)
    in_maps = []
    for c in range(N_CORES):
        i0 = c * IC
        in_maps.append({
            "x": xf,
            "wg": np.ascontiguousarray(w_gate[i0:i0 + IC, :].T,
                                       dtype=np.float32),
            "wu": np.ascontiguousarray(w_up[i0:i0 + IC, :].T,
                                       dtype=np.float32),
            "wd": np.ascontiguousarray(w_down[:, i0:i0 + IC].T,
                                       dtype=np.float32),
            "scales": scales,
        })
    return in_maps


def _assemble_out(results):
    out = np.empty((T, H), dtype=np.float32)
    tpc = TSUP // N_CORES
    for c in range(N_CORES):
        o = results[c]["out_rs"]
        for b in range(NSUP):
            out[b * TSUP + c * tpc: b * TSUP + (c + 1) * tpc] = o[b]
    return out.reshape(B, S, H)


def kernel(x, w_gate, w_up, w_down, s_gate, s_up, s_down):
    from concourse.bass_utils import run_bass_kernel_spmd

    nc = _get_compiled()
    in_maps = _make_in_maps(x, w_gate, w_up, w_down, s_gate, s_up, s_down)
    res = run_bass_kernel_spmd(nc, in_maps, core_ids=list(range(N_CORES)))
    return _assemble_out(res.results)
